# revision 5
# baseline (speedup 1.0000x reference)
"""Causal multi-head attention block (qkv proj + partial RoPE + causal attn +
out proj) for Trainium2, distributed over 8 NeuronCores.

Sharding: core i handles batch b = i//2 and head-group g = i%2 (6 of 12 heads).
Each core computes a partial output projection (contraction over its 6 heads'
384 channels); the host sums the two head-group partials per batch.

v3 design notes (from trace analysis of v2 @336us):
  - PE matmul was 99.5% busy: the kernel is tensor-engine bound. Matmul cost
    on HW = ~173ns SBUF access latency (mostly hidden when back-to-back)
    + cols * 0.417ns. fp32r runs ~1.25 cyc/col and triggers power throttling
    (31% of time capped at 50% util) -> all matmuls now bf16 (1 cyc/col).
  - qk projection merged from 7 M-tiles to 6 (rope r1/r2 rows packed with
    pass rows into full 128-row tiles) - fewer streamed columns.
  - Phases interleaved: attn(jq=jt) -> proj(jt+1) -> outproj(jq=jt), so the
    out-proj normalization latency hides under proj matmuls and PE gaps fill.
  - Attention inner loop software-pipelined with lag 2: scores(kt) issue two
    k-tiles ahead of av(kt) so PE never waits on ACT exp.
  - Softmax normalization per pair without cross-pair gather: denominator row
    (ones-column of v) stays in oun; DVE reciprocal on the single-partition
    slice; broadcast to 128 partitions via two K=1 matmuls (e2); DVE multiply
    into bf16 o_sb consumed by the out-projection.
  - DMA order: x(0) first, then weights chunk-by-chunk; x(jt+1) prefetched in
    a single rearranged DMA at the start of attention(jt).
"""

import numpy as np

B, T, C = 4, 2048, 768
NH, HD, RD = 12, 64, 16
NHL = NH // 2          # heads per core (local)
NPAIR = NHL // 2       # head pairs per core
CL = NHL * HD          # local channels (384)
TQ = 512               # q tile
NTQ = T // TQ
NKT = T // 128         # k tiles of 128

_cache = {}


def _build(debug=False):
    import concourse.bacc as bacc
    import concourse.mybir as mybir
    import concourse.tile as tile

    F32R = mybir.dt.float32r
    F32 = mybir.dt.float32
    BF16 = mybir.dt.bfloat16
    AF = mybir.ActivationFunctionType
    MUL = mybir.AluOpType.mult
    SUB = mybir.AluOpType.subtract
    ADD = mybir.AluOpType.add

    nc = bacc.Bacc(trn_type="TRN2", name="attn8v3")

    xt = nc.dram_tensor("xt", [C, T], BF16, kind="ExternalInput")
    wqkt = nc.dram_tensor("wqkt", [C, 2 * CL], BF16, kind="ExternalInput")
    wvt = nc.dram_tensor("wvt", [C, CL], BF16, kind="ExternalInput")
    wot = nc.dram_tensor("wot", [CL, C], BF16, kind="ExternalInput")
    cosb = nc.dram_tensor("cosb", [96, T], F32, kind="ExternalInput")
    sinb = nc.dram_tensor("sinb", [96, T], F32, kind="ExternalInput")
    tri2 = nc.dram_tensor("tri2", [128, 2 * 128], BF16, kind="ExternalInput")
    e2 = nc.dram_tensor("e2", [1, 2, 128], F32R, kind="ExternalInput")
    out = nc.dram_tensor("out", [C, T], F32, kind="ExternalOutput")

    # qk-projection M-tiles (wqkt column order, host-built):
    #   tile 0 [128] : r1 rows [96] = (q h0..h5 | k h0..h5) x dims 0:8
    #                  + r2a rows [32] = tensor-heads 0..3 x dims 8:16
    #   tile 1 [128] : r2b rows [64] = tensor-heads 4..11 x dims 8:16
    #                  + pass rows 0:64
    #   tiles 2..5   : pass rows 64:576
    # pass row order: for blk 0..5 (q pairs then k pairs):
    #   h_even dims 16:64 (48 rows), h_odd dims 16:64 (48 rows)
    A_ORDER = [4, 5, 6, 7, 8, 9, 10, 11, 0, 1, 2, 3]

    def pass_dest(row):
        blk, r = divmod(row, 96)
        part = 64 * (r // 48) + 16 + (r % 48)
        return blk, part

    with tile.TileContext(nc) as tc:
        with (
            tc.tile_pool(name="persist", bufs=1) as pp,
            tc.tile_pool(name="weights", bufs=1) as wp,
            tc.tile_pool(name="xload", bufs=2) as xlp,
            tc.tile_pool(name="pstage", bufs=2) as psg,
            tc.tile_pool(name="ropet", bufs=1) as rtp,
            tc.tile_pool(name="rots", bufs=2) as rop,
            tc.tile_pool(name="expp", bufs=3) as xpp,
            tc.tile_pool(name="ounp", bufs=3) as onp,
            tc.tile_pool(name="rinvp", bufs=2) as rip,
            tc.tile_pool(name="osbp", bufs=2) as osp,
            tc.tile_pool(name="misc", bufs=2) as msc,
            tc.tile_pool(name="flex", bufs=2, space="PSUM") as flx,
            tc.tile_pool(name="sps", bufs=2, space="PSUM") as sps,
            tc.tile_pool(name="ops", bufs=1, space="PSUM") as ops,
        ):
            qk_sb = pp.tile([128, 2 * NPAIR, T], BF16, tag="qk")
            v_sb = pp.tile([128, NKT, NHL, HD + 1], BF16, tag="v")
            cos_t = pp.tile([96, T], F32, tag="cos")
            sin_t = pp.tile([96, T], F32, tag="sin")
            tri_t = pp.tile([128, 2 * 128], BF16, tag="tri")
            e2_t = pp.tile([1, 2, 128], F32R, tag="e2")

            wqk_t = wp.tile([128, C // 128, 2 * CL], BF16, tag="wqk")
            wv_t = wp.tile([128, C // 128, CL], BF16, tag="wv")
            wo_t = wp.tile([128, NPAIR, C], BF16, tag="wo")

            xr = xt.rearrange("(c p) t -> p c t", p=128)
            wqk_r = wqkt.rearrange("(co p) m -> co p m", p=128)

            def dma_x(jt):
                x_jt = xlp.tile([128, C // 128, TQ], BF16, tag="x")
                nc.sync.dma_start(x_jt, xr[:, :, jt * TQ:(jt + 1) * TQ])
                return x_jt

            # ---- prologue DMAs: x(0) first, then weights ----
            x_tiles = [dma_x(0)]
            for c in range(C // 128):
                nc.sync.dma_start(wqk_t[:, c], wqk_r[c])
            nc.sync.dma_start(cos_t, cosb[:, :])
            nc.sync.dma_start(sin_t, sinb[:, :])
            nc.sync.dma_start(
                wv_t, wvt.rearrange("(co p) m -> p co m", p=128))
            nc.sync.dma_start(tri_t, tri2[:, :])
            nc.sync.dma_start(e2_t, e2[:, :, :])
            nc.sync.dma_start(
                wo_t, wot.rearrange("(po p) m -> p po m", p=128))
            nc.gpsimd.memset(
                v_sb.bitcast(mybir.dt.uint16).rearrange("p a b c -> p (a b c)"),
                0x3F80)  # bf16 1.0 bit pattern

            def proj(jt):
                """qkv projections for t-tile jt into qk_sb / v_sb."""
                ts = slice(jt * TQ, (jt + 1) * TQ)
                x_jt = x_tiles[jt]
                # rope M-tiles (0, 1) share one [128,2,TQ] psum from ops pool
                rps = ops.tile([128, 2, TQ], F32, tag="o")
                for mt in range(2):
                    for c in range(C // 128):
                        nc.tensor.matmul(
                            rps[:, mt], wqk_t[:, c, 128 * mt:128 * (mt + 1)],
                            x_jt[:, c], start=(c == 0), stop=(c == C // 128 - 1))
                # rope: rot1 = r1*cos - r2*sin ; rot2 = r2*cos + r1*sin
                # rope rows use A_ORDER = tensor-heads [4..11, 0..3] so the
                # r2 psum split lands on the 64-partition boundary (DVE APs
                # must be 0/32/64/96-aligned): r1 = rps[0:96, 0];
                # r2 = rps[0:64, 1] (heads 4..11) ++ rps[96:128, 0] (0..3)
                t1 = rtp.tile([96, TQ], F32, tag="t1")
                t2 = rtp.tile([96, TQ], F32, tag="t2")
                rot1 = rop.tile([96, TQ], BF16, tag="rot1")
                rot2 = rop.tile([96, TQ], BF16, tag="rot2")
                nc.vector.tensor_tensor(t1, rps[0:96, 0], cos_t[:, ts], MUL)
                nc.vector.tensor_tensor(
                    t2[0:64], rps[0:64, 1], sin_t[0:64, ts], MUL)
                nc.vector.tensor_tensor(
                    t2[64:96], rps[96:128, 0], sin_t[64:96, ts], MUL)
                nc.vector.tensor_tensor(rot1, t1, t2, SUB)
                t3 = rtp.tile([96, TQ], F32, tag="t1")
                t4 = rtp.tile([96, TQ], F32, tag="t2")
                nc.vector.tensor_tensor(
                    t3[0:64], rps[0:64, 1], cos_t[0:64, ts], MUL)
                nc.vector.tensor_tensor(
                    t3[64:96], rps[96:128, 0], cos_t[64:96, ts], MUL)
                nc.vector.tensor_tensor(t4, rps[0:96, 0], sin_t[:, ts], MUL)
                nc.vector.tensor_tensor(rot2, t3, t4, ADD)
                for i, a in enumerate(A_ORDER):
                    tn, hl = divmod(a, NHL)
                    blk = (0 if tn == 0 else NPAIR) + hl // 2
                    base = 64 * (hl % 2)
                    nc.sync.dma_start(qk_sb[base:base + 8, blk, ts],
                                      rot1[8 * i:8 * i + 8])
                    nc.sync.dma_start(qk_sb[base + 8:base + 16, blk, ts],
                                      rot2[8 * i:8 * i + 8])
                # pass rows: tile 1 rows 64:128 and tiles 2..5
                for mt in range(1, 6):
                    r0 = 0 if mt > 1 else 64          # first pass row in tile
                    prow0 = 128 * mt + r0 - 192       # pass-row index of r0
                    if mt > 1:
                        ps = flx.tile([128, TQ], F32, tag="flex")
                        for c in range(C // 128):
                            nc.tensor.matmul(
                                ps, wqk_t[:, c, 128 * mt:128 * (mt + 1)],
                                x_jt[:, c], start=(c == 0),
                                stop=(c == C // 128 - 1))
                        src = ps
                    else:
                        src = rps[:, 1]
                    stg = psg.tile([128, TQ], BF16, tag="pstg")
                    nc.vector.tensor_copy(stg[r0:128], src[r0:128])
                    row = prow0
                    while row < prow0 + 128 - r0:
                        blk, part = pass_dest(row)
                        run = min(prow0 + 128 - r0 - row, 48 - row % 48)
                        sr = row - prow0 + r0
                        nc.sync.dma_start(
                            qk_sb[part:part + run, blk, ts],
                            stg[sr:sr + run])
                        row += run
                # v projection: pv[tok, chan] per 128-token chunk
                for vt in range(TQ // 128):
                    pvf = flx.tile([128, TQ], F32, tag="flex")
                    pv = pvf[:, 0:CL]
                    kt0 = jt * (TQ // 128) + vt
                    for c in range(C // 128):
                        nc.tensor.matmul(
                            pv, x_jt[:, c, vt * 128:(vt + 1) * 128],
                            wv_t[:, c], start=(c == 0), stop=(c == C // 128 - 1))
                    nc.vector.tensor_copy(
                        v_sb[:, kt0, :, 0:HD],
                        pv.rearrange("p (h d) -> p h d", d=HD))

            def attn(jq):
                """causal attention for q-tile jq; writes o_sb (bf16)."""
                if jq + 1 < NTQ:
                    x_tiles.append(dma_x(jq + 1))
                qs = slice(jq * TQ, (jq + 1) * TQ)
                o_sb = osp.tile([128, NPAIR, TQ], BF16, tag="osb")
                nkt = 4 * (jq + 1)
                for p in range(NPAIR):
                    qb = qk_sb[:, p, qs]
                    kb = qk_sb[:, NPAIR + p, :]
                    o_ps = ops.tile([128, 2, TQ], F32, tag="o")
                    pend = []

                    def issue_av(kt, ep, a):
                        for h in range(2):
                            nc.tensor.matmul(
                                o_ps[0:HD + 1, h, a:TQ],
                                v_sb[:, kt, 2 * p + h, :], ep[:, h, a:TQ],
                                start=(kt == 0), stop=(kt == nkt - 1))

                    for kt in range(nkt):
                        m = kt - 4 * jq
                        a = 0 if m < 0 else 128 * m
                        ks = slice(kt * 128, (kt + 1) * 128)
                        sg = sps.tile([128, 2, TQ], F32, tag="s")
                        nc.tensor.matmul(
                            sg[:, 0, a:TQ], kb[0:64, ks], qb[0:64, a:TQ],
                            start=True, stop=True, tile_position=(0, 0))
                        nc.tensor.matmul(
                            sg[:, 1, a:TQ], kb[64:128, ks], qb[64:128, a:TQ],
                            start=True, stop=True, tile_position=(64, 0))
                        ep = xpp.tile([128, 2, TQ], BF16, tag="e")
                        nc.scalar.activation(ep[:, :, a:TQ], sg[:, :, a:TQ],
                                             AF.Exp, scale=0.125)
                        if m >= 0:
                            nc.gpsimd.tensor_tensor(
                                ep[:, :, a:a + 128], ep[:, :, a:a + 128],
                                tri_t.rearrange("p (h q) -> p h q", h=2), MUL)
                        pend.append((kt, ep, a))
                        if len(pend) > 2:
                            issue_av(*pend.pop(0))
                    for job in pend:
                        issue_av(*job)
                    # per-pair softmax normalization, decoupled from PSUM:
                    # rowsums live in oun row 64 (ones-column of v); DVE
                    # reciprocal on that single-partition slice; two K=1
                    # matmuls broadcast rinv to 128 partitions; DVE multiply.
                    oun = onp.tile([128, 2, TQ], F32, tag="oun")
                    nc.scalar.copy(oun[0:HD + 1], o_ps[0:HD + 1])
                    rinv = rip.tile([1, 2, TQ], F32R, tag="rinv")
                    with nc.allow_low_precision(reason="f32r storage is 32-bit"):
                        nc.vector.reciprocal(rinv, oun[HD:HD + 1])
                    bc = flx.tile([128, TQ], F32, tag="flex")
                    for h in range(2):
                        nc.tensor.matmul(bc, e2_t[:, h], rinv[:, h],
                                         start=(h == 0), stop=(h == 1))
                    nc.vector.tensor_tensor(
                        o_sb[0:64, p], oun[0:64, 0], bc[0:64], MUL)
                    nc.vector.tensor_tensor(
                        o_sb[64:128, p], oun[0:64, 1], bc[64:128], MUL)
                return o_sb

            def outproj(jq, o_sb):
                qs = slice(jq * TQ, (jq + 1) * TQ)
                for dt in range(C // 128):
                    po = flx.tile([128, TQ], F32, tag="flex")
                    for p in range(NPAIR):
                        nc.tensor.matmul(
                            po, wo_t[:, p, dt * 128:(dt + 1) * 128],
                            o_sb[:, p], start=(p == 0), stop=(p == NPAIR - 1))
                    ost = msc.tile([128, TQ], F32, tag="ost")
                    nc.vector.tensor_copy(ost, po)
                    nc.sync.dma_start(
                        out.rearrange("(do p) t -> do p t", p=128)[dt, :, qs], ost)

            proj(0)
            for jt in range(NTQ):
                o_sb = attn(jt)
                if jt + 1 < NTQ:
                    proj(jt + 1)
                outproj(jt, o_sb)

    nc.compile()
    return nc


def _host_inputs(x, w_qkv, w_out):
    """Build per-core input dicts. Core i: batch i//2, head-group i%2."""
    import ml_dtypes

    BF = ml_dtypes.bfloat16
    xf = np.asarray(x, dtype=np.float32)
    w3 = np.asarray(w_qkv, dtype=np.float32).reshape(3, NH, HD, C)
    wo = np.asarray(w_out, dtype=np.float32)

    per_group = []
    for g in range(2):
        hs = range(g * NHL, (g + 1) * NHL)
        A_ORDER = [4, 5, 6, 7, 8, 9, 10, 11, 0, 1, 2, 3]
        rows = []
        # M-tile 0: r1 (A_ORDER tensor-heads x dims 0:8) + r2a (last 4 of
        # A_ORDER x dims 8:16); M-tile 1 rows 0:64: r2b (first 8 x 8:16)
        for a in A_ORDER:
            tn, hl = divmod(a, NHL)
            rows.append(w3[tn, g * NHL + hl, 0:8])
        for a in A_ORDER[8:12]:
            tn, hl = divmod(a, NHL)
            rows.append(w3[tn, g * NHL + hl, 8:16])
        for a in A_ORDER[0:8]:
            tn, hl = divmod(a, NHL)
            rows.append(w3[tn, g * NHL + hl, 8:16])
        # pass rows: blk 0..5 = (q pairs, k pairs); per blk h_even, h_odd
        for blk in range(6):
            tn, pr = divmod(blk, NPAIR)
            for ho in range(2):
                rows.append(w3[tn, g * NHL + 2 * pr + ho, 16:64])
        wqk = np.concatenate(rows, axis=0)                  # [768, C]
        wqkt = np.ascontiguousarray(wqk.T).astype(BF)       # [C, 768]
        wv = w3[2, list(hs)].reshape(CL, C)                 # [384, C]
        wvt = np.ascontiguousarray(wv.T).astype(BF)
        wotr = np.ascontiguousarray(
            wo[:, g * CL:(g + 1) * CL].T).astype(BF)        # [384, 768]
        per_group.append((wqkt, wvt, wotr))

    j = np.arange(RD // 2, dtype=np.float64)
    freqs = 1.0 / (10000.0 ** (2 * j / RD))
    t = np.arange(T, dtype=np.float64)
    ang = t[None, :] * freqs[:, None]                        # [8, T]
    cosb = np.ascontiguousarray(np.tile(np.cos(ang), (12, 1)), dtype=np.float32)
    sinb = np.ascontiguousarray(np.tile(np.sin(ang), (12, 1)), dtype=np.float32)

    kk = np.arange(128)[:, None]
    qq = np.arange(128)[None, :]
    tri = (kk <= qq).astype(BF)
    tri2 = np.ascontiguousarray(np.concatenate([tri, tri], axis=1))
    e2 = np.zeros((1, 2, 128), dtype=np.float32)
    e2[0, 0, 0:64] = 1.0
    e2[0, 1, 64:128] = 1.0

    in_maps = []
    for i in range(8):
        b, g = divmod(i, 2)
        wqkt, wvt, wotr = per_group[g]
        in_maps.append({
            "xt": np.ascontiguousarray(xf[b].T).astype(BF),
            "wqkt": wqkt, "wvt": wvt, "wot": wotr,
            "cosb": cosb, "sinb": sinb, "tri2": tri2, "e2": e2,
        })
    return in_maps


def kernel(x, w_qkv, w_out, _trace=False):
    from concourse.bass_utils import run_bass_kernel_spmd

    if "nc" not in _cache:
        _cache["nc"] = _build()
    nc = _cache["nc"]
    in_maps = _host_inputs(x, w_qkv, w_out)
    res = run_bass_kernel_spmd(nc, in_maps, core_ids=list(range(8)),
                               trace=_trace)
    _cache["last_result"] = res
    out = np.empty((B, T, C), dtype=np.float32)
    for b in range(B):
        acc = res.results[2 * b]["out"].astype(np.float32) + \
            res.results[2 * b + 1]["out"].astype(np.float32)
        out[b] = acc.T
    return out


# revision 6
# speedup vs baseline: 1.1744x; 1.1744x over previous
"""Causal multi-head attention block (qkv proj + partial RoPE + causal attn +
out proj) for Trainium2, distributed over 8 NeuronCores.

Sharding: core i handles batch b = i//2 and head-group g = i%2 (6 of 12 heads).
Each core computes a partial output projection (contraction over its 6 heads'
384 channels); the host sums the two head-group partials per batch.

v3 design notes (from trace analysis of v2 @336us):
  - PE matmul was 99.5% busy: the kernel is tensor-engine bound. Matmul cost
    on HW = ~173ns SBUF access latency (mostly hidden when back-to-back)
    + cols * 0.417ns. fp32r runs ~1.25 cyc/col and triggers power throttling
    (31% of time capped at 50% util) -> all matmuls now bf16 (1 cyc/col).
  - qk projection merged from 7 M-tiles to 6 (rope r1/r2 rows packed with
    pass rows into full 128-row tiles) - fewer streamed columns.
  - Phases interleaved: attn(jq=jt) -> proj(jt+1) -> outproj(jq=jt), so the
    out-proj normalization latency hides under proj matmuls and PE gaps fill.
  - Attention inner loop software-pipelined with lag 2: scores(kt) issue two
    k-tiles ahead of av(kt) so PE never waits on ACT exp.
  - Softmax normalization per pair without cross-pair gather: denominator row
    (ones-column of v) stays in oun; DVE reciprocal on the single-partition
    slice; broadcast to 128 partitions via two K=1 matmuls (e2); DVE multiply
    into bf16 o_sb consumed by the out-projection.
  - DMA order: x(0) first, then weights chunk-by-chunk; x(jt+1) prefetched in
    a single rearranged DMA at the start of attention(jt).
"""

import numpy as np

B, T, C = 4, 2048, 768
NH, HD, RD = 12, 64, 16
NHL = NH // 2          # heads per core (local)
NPAIR = NHL // 2       # head pairs per core
CL = NHL * HD          # local channels (384)
TQ = 512               # q tile
NTQ = T // TQ
NKT = T // 128         # k tiles of 128

_cache = {}


def _build(debug=False):
    import concourse.bacc as bacc
    import concourse.mybir as mybir
    import concourse.tile as tile

    F32R = mybir.dt.float32r
    F32 = mybir.dt.float32
    BF16 = mybir.dt.bfloat16
    AF = mybir.ActivationFunctionType
    MUL = mybir.AluOpType.mult
    SUB = mybir.AluOpType.subtract
    ADD = mybir.AluOpType.add

    nc = bacc.Bacc(trn_type="TRN2", name="attn8v3")

    xt = nc.dram_tensor("xt", [C, T], BF16, kind="ExternalInput")
    wqkt = nc.dram_tensor("wqkt", [C, 2 * CL], BF16, kind="ExternalInput")
    wvt = nc.dram_tensor("wvt", [C, CL], BF16, kind="ExternalInput")
    wot = nc.dram_tensor("wot", [CL, C], BF16, kind="ExternalInput")
    cosb = nc.dram_tensor("cosb", [96, T], F32, kind="ExternalInput")
    sinb = nc.dram_tensor("sinb", [96, T], F32, kind="ExternalInput")
    tri2 = nc.dram_tensor("tri2", [128, 2 * 128], BF16, kind="ExternalInput")
    e6 = nc.dram_tensor("e6", [6, NPAIR * 128], F32R, kind="ExternalInput")
    out = nc.dram_tensor("out", [C, T], F32, kind="ExternalOutput")

    # qk-projection M-tiles (wqkt column order, host-built):
    #   tile 0 [128] : r1 rows [96] = (q h0..h5 | k h0..h5) x dims 0:8
    #                  + r2a rows [32] = tensor-heads 0..3 x dims 8:16
    #   tile 1 [128] : r2b rows [64] = tensor-heads 4..11 x dims 8:16
    #                  + pass rows 0:64
    #   tiles 2..5   : pass rows 64:576
    # pass row order: for blk 0..5 (q pairs then k pairs):
    #   h_even dims 16:64 (48 rows), h_odd dims 16:64 (48 rows)
    A_ORDER = [4, 5, 6, 7, 8, 9, 10, 11, 0, 1, 2, 3]

    def pass_dest(row):
        blk, r = divmod(row, 96)
        part = 64 * (r // 48) + 16 + (r % 48)
        return blk, part

    with tile.TileContext(nc) as tc:
        with (
            tc.tile_pool(name="persist", bufs=1) as pp,
            tc.tile_pool(name="weights", bufs=1) as wp,
            tc.tile_pool(name="xload", bufs=2) as xlp,
            tc.tile_pool(name="pstage", bufs=2) as psg,
            tc.tile_pool(name="ropet", bufs=1) as rtp,
            tc.tile_pool(name="rots", bufs=2) as rop,
            tc.tile_pool(name="expp", bufs=3) as xpp,
            tc.tile_pool(name="ounp", bufs=3) as onp,
            tc.tile_pool(name="rinvp", bufs=2) as rip,
            tc.tile_pool(name="osbp", bufs=2) as osp,
            tc.tile_pool(name="misc", bufs=2) as msc,
            tc.tile_pool(name="flex", bufs=2, space="PSUM") as flx,
            tc.tile_pool(name="sps", bufs=2, space="PSUM") as sps,
            tc.tile_pool(name="ops", bufs=1, space="PSUM") as ops,
        ):
            qk_sb = pp.tile([128, 2 * NPAIR, T], BF16, tag="qk")
            v_sb = pp.tile([128, NKT, NHL, HD + 1], BF16, tag="v")
            cos_t = pp.tile([96, T], F32, tag="cos")
            sin_t = pp.tile([96, T], F32, tag="sin")
            tri_t = pp.tile([128, 2 * 128], BF16, tag="tri")
            e6_t = pp.tile([6, NPAIR * 128], F32R, tag="e6")
            rs6_t = pp.tile([6, TQ], F32, tag="rs6")
            rinv6_t = pp.tile([6, TQ], F32R, tag="rinv6")

            wqk_t = wp.tile([128, C // 128, 2 * CL], BF16, tag="wqk")
            wv_t = wp.tile([128, C // 128, CL], BF16, tag="wv")
            wo_t = wp.tile([128, NPAIR, C], BF16, tag="wo")

            xr = xt.rearrange("(c p) t -> p c t", p=128)
            wqk_r = wqkt.rearrange("(co p) m -> co p m", p=128)

            def dma_x(jt):
                x_jt = xlp.tile([128, C // 128, TQ], BF16, tag="x")
                ts = slice(jt * TQ, (jt + 1) * TQ)
                nc.sync.dma_start(x_jt[:, 0:3], xr[:, 0:3, ts])
                nc.sync.dma_start(x_jt[:, 3:6], xr[:, 3:6, ts])
                return x_jt

            # ---- prologue DMAs: x(0) first, then weights ----
            x_tiles = [dma_x(0)]
            for c in range(C // 128):
                nc.sync.dma_start(wqk_t[:, c], wqk_r[c])
            nc.sync.dma_start(
                wv_t, wvt.rearrange("(co p) m -> p co m", p=128))
            nc.sync.dma_start(cos_t, cosb[:, :])
            nc.sync.dma_start(sin_t, sinb[:, :])
            nc.sync.dma_start(tri_t, tri2[:, :])
            nc.sync.dma_start(e6_t, e6[:, :])
            nc.sync.dma_start(
                wo_t, wot.rearrange("(po p) m -> p po m", p=128))
            nc.gpsimd.memset(
                v_sb.bitcast(mybir.dt.uint16).rearrange("p a b c -> p (a b c)"),
                0x3F80)  # bf16 1.0 bit pattern

            def proj(jt):
                """qkv projections for t-tile jt into qk_sb / v_sb."""
                ts = slice(jt * TQ, (jt + 1) * TQ)
                x_jt = x_tiles[jt]
                # rope M-tiles (0, 1) share one [128,2,TQ] psum from ops pool
                rps = ops.tile([128, 2, TQ], F32, tag="o")
                for mt in range(2):
                    for c in range(C // 128):
                        nc.tensor.matmul(
                            rps[:, mt], wqk_t[:, c, 128 * mt:128 * (mt + 1)],
                            x_jt[:, c], start=(c == 0), stop=(c == C // 128 - 1))
                # rope: rot1 = r1*cos - r2*sin ; rot2 = r2*cos + r1*sin
                # rope rows use A_ORDER = tensor-heads [4..11, 0..3] so the
                # r2 psum split lands on the 64-partition boundary (DVE APs
                # must be 0/32/64/96-aligned): r1 = rps[0:96, 0];
                # r2 = rps[0:64, 1] (heads 4..11) ++ rps[96:128, 0] (0..3)
                t1 = rtp.tile([96, TQ], F32, tag="t1")
                t2 = rtp.tile([96, TQ], F32, tag="t2")
                rot1 = rop.tile([96, TQ], BF16, tag="rot1")
                rot2 = rop.tile([96, TQ], BF16, tag="rot2")
                nc.vector.tensor_tensor(t1, rps[0:96, 0], cos_t[:, ts], MUL)
                nc.vector.tensor_tensor(
                    t2[0:64], rps[0:64, 1], sin_t[0:64, ts], MUL)
                nc.vector.tensor_tensor(
                    t2[64:96], rps[96:128, 0], sin_t[64:96, ts], MUL)
                nc.vector.tensor_tensor(rot1, t1, t2, SUB)
                t3 = rtp.tile([96, TQ], F32, tag="t1")
                t4 = rtp.tile([96, TQ], F32, tag="t2")
                nc.vector.tensor_tensor(
                    t3[0:64], rps[0:64, 1], cos_t[0:64, ts], MUL)
                nc.vector.tensor_tensor(
                    t3[64:96], rps[96:128, 0], cos_t[64:96, ts], MUL)
                nc.vector.tensor_tensor(t4, rps[0:96, 0], sin_t[:, ts], MUL)
                nc.vector.tensor_tensor(rot2, t3, t4, ADD)
                for i, a in enumerate(A_ORDER):
                    tn, hl = divmod(a, NHL)
                    blk = (0 if tn == 0 else NPAIR) + hl // 2
                    base = 64 * (hl % 2)
                    nc.sync.dma_start(qk_sb[base:base + 8, blk, ts],
                                      rot1[8 * i:8 * i + 8])
                    nc.sync.dma_start(qk_sb[base + 8:base + 16, blk, ts],
                                      rot2[8 * i:8 * i + 8])
                # pass rows: tile 1 rows 64:128 and tiles 2..5
                for mt in range(1, 6):
                    r0 = 0 if mt > 1 else 64          # first pass row in tile
                    prow0 = 128 * mt + r0 - 192       # pass-row index of r0
                    if mt > 1:
                        ps = flx.tile([128, TQ], F32, tag="flex")
                        for c in range(C // 128):
                            nc.tensor.matmul(
                                ps, wqk_t[:, c, 128 * mt:128 * (mt + 1)],
                                x_jt[:, c], start=(c == 0),
                                stop=(c == C // 128 - 1))
                        src = ps
                    else:
                        src = rps[:, 1]
                    stg = psg.tile([128, TQ], BF16, tag="pstg")
                    nc.vector.tensor_copy(stg[r0:128], src[r0:128])
                    row = prow0
                    while row < prow0 + 128 - r0:
                        blk, part = pass_dest(row)
                        run = min(prow0 + 128 - r0 - row, 48 - row % 48)
                        sr = row - prow0 + r0
                        nc.sync.dma_start(
                            qk_sb[part:part + run, blk, ts],
                            stg[sr:sr + run])
                        row += run
                # v projection: pv[tok, chan] per 128-token chunk
                for vt in range(TQ // 128):
                    pvf = flx.tile([128, TQ], F32, tag="flex")
                    pv = pvf[:, 0:CL]
                    kt0 = jt * (TQ // 128) + vt
                    for c in range(C // 128):
                        nc.tensor.matmul(
                            pv, x_jt[:, c, vt * 128:(vt + 1) * 128],
                            wv_t[:, c], start=(c == 0), stop=(c == C // 128 - 1))
                    nc.vector.tensor_copy(
                        v_sb[:, kt0, :, 0:HD],
                        pv.rearrange("p (h d) -> p h d", d=HD))

            def attn(jq):
                """causal attention for q-tile jq; writes o_sb (bf16)."""
                if jq + 1 < NTQ:
                    x_tiles.append(dma_x(jq + 1))
                qs = slice(jq * TQ, (jq + 1) * TQ)
                o_sb = osp.tile([128, NPAIR, TQ], BF16, tag="osb")
                ouns = []
                nkt = 4 * (jq + 1)
                for p in range(NPAIR):
                    qb = qk_sb[:, p, qs]
                    kb = qk_sb[:, NPAIR + p, :]
                    o_ps = ops.tile([128, 2, TQ], F32, tag="o")
                    pend = []

                    def issue_av(kt, ep, a):
                        for h in range(2):
                            nc.tensor.matmul(
                                o_ps[0:HD + 1, h, a:TQ],
                                v_sb[:, kt, 2 * p + h, :], ep[:, h, a:TQ],
                                start=(kt == 0), stop=(kt == nkt - 1))

                    for kt in range(nkt):
                        m = kt - 4 * jq
                        a = 0 if m < 0 else 128 * m
                        ks = slice(kt * 128, (kt + 1) * 128)
                        sg = sps.tile([128, 2, TQ], F32, tag="s")
                        nc.tensor.matmul(
                            sg[:, 0, a:TQ], kb[0:64, ks], qb[0:64, a:TQ],
                            start=True, stop=True, tile_position=(0, 0))
                        nc.tensor.matmul(
                            sg[:, 1, a:TQ], kb[64:128, ks], qb[64:128, a:TQ],
                            start=True, stop=True, tile_position=(64, 0))
                        ep = xpp.tile([128, 2, TQ], BF16, tag="e")
                        nc.scalar.activation(ep[:, :, a:TQ], sg[:, :, a:TQ],
                                             AF.Exp, scale=0.125)
                        if m >= 0:
                            nc.gpsimd.tensor_tensor(
                                ep[:, :, a:a + 128], ep[:, :, a:a + 128],
                                tri_t.rearrange("p (h q) -> p h q", h=2), MUL)
                        pend.append((kt, ep, a))
                        if len(pend) > 2:
                            issue_av(*pend.pop(0))
                    for job in pend:
                        issue_av(*job)
                    # softmax rowsums (ones-column of v) -> oun row 64;
                    # gather to rs6 per pair (cheap 2-descriptor DMA) so the
                    # slow DVE reciprocal runs ONCE per jq on [6, TQ] (free
                    # size, not partition count, sets reciprocal cost).
                    oun = onp.tile([128, 2, TQ], F32, tag="oun")
                    nc.scalar.copy(oun[0:HD + 1], o_ps[0:HD + 1])
                    nc.sync.dma_start(rs6_t[2 * p:2 * p + 2, :],
                                      oun[HD:HD + 1, :, :])
                    ouns.append(oun)
                with nc.allow_low_precision(reason="f32r storage is 32-bit"):
                    nc.vector.reciprocal(rinv6_t, rs6_t)
                for p in range(NPAIR):
                    bc = flx.tile([128, TQ], F32, tag="flex")
                    nc.tensor.matmul(bc, e6_t[:, p * 128:(p + 1) * 128],
                                     rinv6_t, start=True, stop=True)
                    oun = ouns[p]
                    nc.vector.tensor_tensor(
                        o_sb[0:64, p], oun[0:64, 0], bc[0:64], MUL)
                    nc.vector.tensor_tensor(
                        o_sb[64:128, p], oun[0:64, 1], bc[64:128], MUL)
                return o_sb

            def outproj(jq, o_sb):
                qs = slice(jq * TQ, (jq + 1) * TQ)
                for dt in range(C // 128):
                    po = flx.tile([128, TQ], F32, tag="flex")
                    for p in range(NPAIR):
                        nc.tensor.matmul(
                            po, wo_t[:, p, dt * 128:(dt + 1) * 128],
                            o_sb[:, p], start=(p == 0), stop=(p == NPAIR - 1))
                    ost = msc.tile([128, TQ], F32, tag="ost")
                    nc.vector.tensor_copy(ost, po)
                    nc.sync.dma_start(
                        out.rearrange("(do p) t -> do p t", p=128)[dt, :, qs], ost)

            proj(0)
            for jt in range(NTQ):
                o_sb = attn(jt)
                if jt + 1 < NTQ:
                    proj(jt + 1)
                outproj(jt, o_sb)

    nc.compile()
    return nc


def _host_inputs(x, w_qkv, w_out):
    """Build per-core input dicts. Core i: batch i//2, head-group i%2."""
    import ml_dtypes

    BF = ml_dtypes.bfloat16
    xf = np.asarray(x, dtype=np.float32)
    w3 = np.asarray(w_qkv, dtype=np.float32).reshape(3, NH, HD, C)
    wo = np.asarray(w_out, dtype=np.float32)

    per_group = []
    for g in range(2):
        hs = range(g * NHL, (g + 1) * NHL)
        A_ORDER = [4, 5, 6, 7, 8, 9, 10, 11, 0, 1, 2, 3]
        rows = []
        # M-tile 0: r1 (A_ORDER tensor-heads x dims 0:8) + r2a (last 4 of
        # A_ORDER x dims 8:16); M-tile 1 rows 0:64: r2b (first 8 x 8:16)
        for a in A_ORDER:
            tn, hl = divmod(a, NHL)
            rows.append(w3[tn, g * NHL + hl, 0:8])
        for a in A_ORDER[8:12]:
            tn, hl = divmod(a, NHL)
            rows.append(w3[tn, g * NHL + hl, 8:16])
        for a in A_ORDER[0:8]:
            tn, hl = divmod(a, NHL)
            rows.append(w3[tn, g * NHL + hl, 8:16])
        # pass rows: blk 0..5 = (q pairs, k pairs); per blk h_even, h_odd
        for blk in range(6):
            tn, pr = divmod(blk, NPAIR)
            for ho in range(2):
                rows.append(w3[tn, g * NHL + 2 * pr + ho, 16:64])
        wqk = np.concatenate(rows, axis=0)                  # [768, C]
        wqkt = np.ascontiguousarray(wqk.T).astype(BF)       # [C, 768]
        wv = w3[2, list(hs)].reshape(CL, C)                 # [384, C]
        wvt = np.ascontiguousarray(wv.T).astype(BF)
        wotr = np.ascontiguousarray(
            wo[:, g * CL:(g + 1) * CL].T).astype(BF)        # [384, 768]
        per_group.append((wqkt, wvt, wotr))

    j = np.arange(RD // 2, dtype=np.float64)
    freqs = 1.0 / (10000.0 ** (2 * j / RD))
    t = np.arange(T, dtype=np.float64)
    ang = t[None, :] * freqs[:, None]                        # [8, T]
    cosb = np.ascontiguousarray(np.tile(np.cos(ang), (12, 1)), dtype=np.float32)
    sinb = np.ascontiguousarray(np.tile(np.sin(ang), (12, 1)), dtype=np.float32)

    kk = np.arange(128)[:, None]
    qq = np.arange(128)[None, :]
    tri = (kk <= qq).astype(BF)
    tri2 = np.ascontiguousarray(np.concatenate([tri, tri], axis=1))
    e6 = np.zeros((6, NPAIR * 128), dtype=np.float32)
    for p in range(NPAIR):
        e6[2 * p, p * 128:p * 128 + 64] = 1.0
        e6[2 * p + 1, p * 128 + 64:(p + 1) * 128] = 1.0

    in_maps = []
    for i in range(8):
        b, g = divmod(i, 2)
        wqkt, wvt, wotr = per_group[g]
        in_maps.append({
            "xt": np.ascontiguousarray(xf[b].T).astype(BF),
            "wqkt": wqkt, "wvt": wvt, "wot": wotr,
            "cosb": cosb, "sinb": sinb, "tri2": tri2, "e6": e6,
        })
    return in_maps


def kernel(x, w_qkv, w_out, _trace=False):
    from concourse.bass_utils import run_bass_kernel_spmd

    if "nc" not in _cache:
        _cache["nc"] = _build()
    nc = _cache["nc"]
    in_maps = _host_inputs(x, w_qkv, w_out)
    res = run_bass_kernel_spmd(nc, in_maps, core_ids=list(range(8)),
                               trace=_trace)
    _cache["last_result"] = res
    out = np.empty((B, T, C), dtype=np.float32)
    for b in range(B):
        acc = res.results[2 * b]["out"].astype(np.float32) + \
            res.results[2 * b + 1]["out"].astype(np.float32)
        out[b] = acc.T
    return out


# revision 7
# speedup vs baseline: 1.2087x; 1.0292x over previous
"""Causal multi-head attention block (qkv proj + partial RoPE + causal attn +
out proj) for Trainium2, distributed over 8 NeuronCores.

Sharding: core i handles batch b = i//2 and head-group g = i%2 (6 of 12 heads).
Each core computes a partial output projection (contraction over its 6 heads'
384 channels); the host sums the two head-group partials per batch.

v3 design notes (from trace analysis of v2 @336us):
  - PE matmul was 99.5% busy: the kernel is tensor-engine bound. Matmul cost
    on HW = ~173ns SBUF access latency (mostly hidden when back-to-back)
    + cols * 0.417ns. fp32r runs ~1.25 cyc/col and triggers power throttling
    (31% of time capped at 50% util) -> all matmuls now bf16 (1 cyc/col).
  - qk projection merged from 7 M-tiles to 6 (rope r1/r2 rows packed with
    pass rows into full 128-row tiles) - fewer streamed columns.
  - Phases interleaved: attn(jq=jt) -> proj(jt+1) -> outproj(jq=jt), so the
    out-proj normalization latency hides under proj matmuls and PE gaps fill.
  - Attention inner loop software-pipelined with lag 2: scores(kt) issue two
    k-tiles ahead of av(kt) so PE never waits on ACT exp.
  - Softmax normalization per pair without cross-pair gather: denominator row
    (ones-column of v) stays in oun; DVE reciprocal on the single-partition
    slice; broadcast to 128 partitions via two K=1 matmuls (e2); DVE multiply
    into bf16 o_sb consumed by the out-projection.
  - DMA order: x(0) first, then weights chunk-by-chunk; x(jt+1) prefetched in
    a single rearranged DMA at the start of attention(jt).
"""

import numpy as np

B, T, C = 4, 2048, 768
NH, HD, RD = 12, 64, 16
NHL = NH // 2          # heads per core (local)
NPAIR = NHL // 2       # head pairs per core
CL = NHL * HD          # local channels (384)
TQ = 512               # q tile
NTQ = T // TQ
NKT = T // 128         # k tiles of 128

_cache = {}


def _build(debug=False):
    import concourse.bacc as bacc
    import concourse.mybir as mybir
    import concourse.tile as tile

    F32R = mybir.dt.float32r
    F32 = mybir.dt.float32
    BF16 = mybir.dt.bfloat16
    AF = mybir.ActivationFunctionType
    MUL = mybir.AluOpType.mult
    SUB = mybir.AluOpType.subtract
    ADD = mybir.AluOpType.add

    nc = bacc.Bacc(trn_type="TRN2", name="attn8v3")

    xt = nc.dram_tensor("xt", [C, T], BF16, kind="ExternalInput")
    wqkt = nc.dram_tensor("wqkt", [C, 2 * CL], BF16, kind="ExternalInput")
    wvt = nc.dram_tensor("wvt", [C, CL], BF16, kind="ExternalInput")
    wot = nc.dram_tensor("wot", [CL, C], BF16, kind="ExternalInput")
    cosb = nc.dram_tensor("cosb", [96, T], F32, kind="ExternalInput")
    sinb = nc.dram_tensor("sinb", [96, T], F32, kind="ExternalInput")
    tri2 = nc.dram_tensor("tri2", [128, 2 * 128], BF16, kind="ExternalInput")
    e6 = nc.dram_tensor("e6", [6, NPAIR * 128], F32R, kind="ExternalInput")
    out = nc.dram_tensor("out", [C, T], F32, kind="ExternalOutput")

    # qk-projection M-tiles (wqkt column order, host-built):
    #   tile 0 [128] : r1 rows [96] = (q h0..h5 | k h0..h5) x dims 0:8
    #                  + r2a rows [32] = tensor-heads 0..3 x dims 8:16
    #   tile 1 [128] : r2b rows [64] = tensor-heads 4..11 x dims 8:16
    #                  + pass rows 0:64
    #   tiles 2..5   : pass rows 64:576
    # pass row order: for blk 0..5 (q pairs then k pairs):
    #   h_even dims 16:64 (48 rows), h_odd dims 16:64 (48 rows)
    A_ORDER = [4, 5, 6, 7, 8, 9, 10, 11, 0, 1, 2, 3]

    BLK_ORDER = [0, 3, 1, 4, 2, 5]

    def pass_dest(row):
        g, r = divmod(row, 96)
        part = 64 * (r // 48) + 16 + (r % 48)
        return BLK_ORDER[g], part

    with tile.TileContext(nc) as tc:
        with (
            tc.tile_pool(name="persist", bufs=1) as pp,
            tc.tile_pool(name="weights", bufs=1) as wp,
            tc.tile_pool(name="xload", bufs=2) as xlp,
            tc.tile_pool(name="pstage", bufs=2) as psg,
            tc.tile_pool(name="ropet", bufs=1) as rtp,
            tc.tile_pool(name="rots", bufs=2) as rop,
            tc.tile_pool(name="expp", bufs=3) as xpp,
            tc.tile_pool(name="ounp", bufs=3) as onp,
            tc.tile_pool(name="rinvp", bufs=2) as rip,
            tc.tile_pool(name="osbp", bufs=2) as osp,
            tc.tile_pool(name="misc", bufs=2) as msc,
            tc.tile_pool(name="flex", bufs=2, space="PSUM") as flx,
            tc.tile_pool(name="sps", bufs=2, space="PSUM") as sps,
            tc.tile_pool(name="ops", bufs=1, space="PSUM") as ops,
        ):
            qk_sb = pp.tile([128, 2 * NPAIR, T], BF16, tag="qk")
            v_sb = pp.tile([128, NKT, NHL, HD + 1], BF16, tag="v")
            cos_t = pp.tile([96, T], F32, tag="cos")
            sin_t = pp.tile([96, T], F32, tag="sin")
            tri_t = pp.tile([128, 2 * 128], BF16, tag="tri")
            e6_t = pp.tile([6, NPAIR * 128], F32R, tag="e6")
            rs6_t = pp.tile([6, TQ], F32, tag="rs6")
            rinv6_t = pp.tile([6, TQ], F32R, tag="rinv6")

            wqk_t = wp.tile([128, C // 128, 2 * CL], BF16, tag="wqk")
            wv_t = wp.tile([128, C // 128, CL], BF16, tag="wv")
            wo_t = wp.tile([128, NPAIR, C], BF16, tag="wo")

            xr = xt.rearrange("(c p) t -> p c t", p=128)
            wqk_r = wqkt.rearrange("(co p) m -> co p m", p=128)

            def dma_x(jt):
                x_jt = xlp.tile([128, C // 128, TQ], BF16, tag="x")
                ts = slice(jt * TQ, (jt + 1) * TQ)
                nc.sync.dma_start(x_jt[:, 0:3], xr[:, 0:3, ts])
                nc.sync.dma_start(x_jt[:, 3:6], xr[:, 3:6, ts])
                return x_jt

            # ---- prologue DMAs: x(0) first, then weights ----
            x_tiles = [dma_x(0)]
            for c in range(C // 128):
                nc.sync.dma_start(wqk_t[:, c], wqk_r[c])
            nc.sync.dma_start(
                wv_t, wvt.rearrange("(co p) m -> p co m", p=128))
            nc.sync.dma_start(cos_t, cosb[:, :])
            nc.sync.dma_start(sin_t, sinb[:, :])
            nc.sync.dma_start(tri_t, tri2[:, :])
            nc.sync.dma_start(e6_t, e6[:, :])
            nc.sync.dma_start(
                wo_t, wot.rearrange("(po p) m -> p po m", p=128))
            nc.gpsimd.memset(
                v_sb.bitcast(mybir.dt.uint16).rearrange("p a b c -> p (a b c)"),
                0x3F80)  # bf16 1.0 bit pattern

            def proj(jt):
                """qkv projections for t-tile jt into qk_sb / v_sb."""
                if jt + 1 < NTQ:
                    x_tiles.append(dma_x(jt + 1))
                ts = slice(jt * TQ, (jt + 1) * TQ)
                x_jt = x_tiles[jt]
                # rope M-tiles (0, 1) share one [128,2,TQ] psum from ops pool
                rps = ops.tile([128, 2, TQ], F32, tag="o")
                for mt in range(2):
                    for c in range(C // 128):
                        nc.tensor.matmul(
                            rps[:, mt], wqk_t[:, c, 128 * mt:128 * (mt + 1)],
                            x_jt[:, c], start=(c == 0), stop=(c == C // 128 - 1))
                # rope: rot1 = r1*cos - r2*sin ; rot2 = r2*cos + r1*sin
                # rope rows use A_ORDER = tensor-heads [4..11, 0..3] so the
                # r2 psum split lands on the 64-partition boundary (DVE APs
                # must be 0/32/64/96-aligned): r1 = rps[0:96, 0];
                # r2 = rps[0:64, 1] (heads 4..11) ++ rps[96:128, 0] (0..3)
                t1 = rtp.tile([96, TQ], F32, tag="t1")
                t2 = rtp.tile([96, TQ], F32, tag="t2")
                rot1 = rop.tile([96, TQ], BF16, tag="rot1")
                rot2 = rop.tile([96, TQ], BF16, tag="rot2")
                nc.vector.tensor_tensor(t1, rps[0:96, 0], cos_t[:, ts], MUL)
                nc.vector.tensor_tensor(
                    t2[0:64], rps[0:64, 1], sin_t[0:64, ts], MUL)
                nc.vector.tensor_tensor(
                    t2[64:96], rps[96:128, 0], sin_t[64:96, ts], MUL)
                nc.vector.tensor_tensor(rot1, t1, t2, SUB)
                t3 = rtp.tile([96, TQ], F32, tag="t1")
                t4 = rtp.tile([96, TQ], F32, tag="t2")
                nc.vector.tensor_tensor(
                    t3[0:64], rps[0:64, 1], cos_t[0:64, ts], MUL)
                nc.vector.tensor_tensor(
                    t3[64:96], rps[96:128, 0], cos_t[64:96, ts], MUL)
                nc.vector.tensor_tensor(t4, rps[0:96, 0], sin_t[:, ts], MUL)
                nc.vector.tensor_tensor(rot2, t3, t4, ADD)
                # scatter pair-0 blocks first so attn(jq) p0 unblocks asap
                for a in (0, 1, 6, 7, 2, 3, 8, 9, 4, 5, 10, 11):
                    i = A_ORDER.index(a)
                    tn, hl = divmod(a, NHL)
                    blk = (0 if tn == 0 else NPAIR) + hl // 2
                    base = 64 * (hl % 2)
                    nc.sync.dma_start(qk_sb[base:base + 8, blk, ts],
                                      rot1[8 * i:8 * i + 8])
                    nc.sync.dma_start(qk_sb[base + 8:base + 16, blk, ts],
                                      rot2[8 * i:8 * i + 8])
                # pass rows: tile 1 rows 64:128 and tiles 2..5
                for mt in range(1, 6):
                    r0 = 0 if mt > 1 else 64          # first pass row in tile
                    prow0 = 128 * mt + r0 - 192       # pass-row index of r0
                    if mt > 1:
                        ps = flx.tile([128, TQ], F32, tag="flex")
                        for c in range(C // 128):
                            nc.tensor.matmul(
                                ps, wqk_t[:, c, 128 * mt:128 * (mt + 1)],
                                x_jt[:, c], start=(c == 0),
                                stop=(c == C // 128 - 1))
                        src = ps
                    else:
                        src = rps[:, 1]
                    stg = psg.tile([128, TQ], BF16, tag="pstg")
                    nc.vector.tensor_copy(stg[r0:128], src[r0:128])
                    row = prow0
                    while row < prow0 + 128 - r0:
                        blk, part = pass_dest(row)
                        run = min(prow0 + 128 - r0 - row, 48 - row % 48)
                        sr = row - prow0 + r0
                        nc.sync.dma_start(
                            qk_sb[part:part + run, blk, ts],
                            stg[sr:sr + run])
                        row += run
                # v projection: pv[tok, chan] per 128-token chunk
                for vt in range(TQ // 128):
                    pvf = flx.tile([128, TQ], F32, tag="flex")
                    pv = pvf[:, 0:CL]
                    kt0 = jt * (TQ // 128) + vt
                    for c in range(C // 128):
                        nc.tensor.matmul(
                            pv, x_jt[:, c, vt * 128:(vt + 1) * 128],
                            wv_t[:, c], start=(c == 0), stop=(c == C // 128 - 1))
                    nc.vector.tensor_copy(
                        v_sb[:, kt0, :, 0:HD],
                        pv.rearrange("p (h d) -> p h d", d=HD))

            def attn(jq):
                """causal attention for q-tile jq; writes o_sb (bf16)."""
                qs = slice(jq * TQ, (jq + 1) * TQ)
                o_sb = osp.tile([128, NPAIR, TQ], BF16, tag="osb")
                ouns = []
                nkt = 4 * (jq + 1)
                for p in range(NPAIR):
                    qb = qk_sb[:, p, qs]
                    kb = qk_sb[:, NPAIR + p, :]
                    o_ps = ops.tile([128, 2, TQ], F32, tag="o")
                    pend = []

                    def issue_av(kt, ep, a):
                        for h in range(2):
                            nc.tensor.matmul(
                                o_ps[0:HD + 1, h, a:TQ],
                                v_sb[:, kt, 2 * p + h, :], ep[:, h, a:TQ],
                                start=(kt == 0), stop=(kt == nkt - 1))

                    for kt in range(nkt):
                        m = kt - 4 * jq
                        a = 0 if m < 0 else 128 * m
                        ks = slice(kt * 128, (kt + 1) * 128)
                        sg = sps.tile([128, 2, TQ], F32, tag="s")
                        nc.tensor.matmul(
                            sg[:, 0, a:TQ], kb[0:64, ks], qb[0:64, a:TQ],
                            start=True, stop=True, tile_position=(0, 0))
                        nc.tensor.matmul(
                            sg[:, 1, a:TQ], kb[64:128, ks], qb[64:128, a:TQ],
                            start=True, stop=True, tile_position=(64, 0))
                        ep = xpp.tile([128, 2, TQ], BF16, tag="e")
                        nc.scalar.activation(ep[:, :, a:TQ], sg[:, :, a:TQ],
                                             AF.Exp, scale=0.125)
                        if m >= 0:
                            nc.gpsimd.tensor_tensor(
                                ep[:, :, a:a + 128], ep[:, :, a:a + 128],
                                tri_t.rearrange("p (h q) -> p h q", h=2), MUL)
                        pend.append((kt, ep, a))
                        if len(pend) > 2:
                            issue_av(*pend.pop(0))
                    for job in pend:
                        issue_av(*job)
                    # softmax rowsums (ones-column of v) -> oun row 64;
                    # gather to rs6 per pair (cheap 2-descriptor DMA) so the
                    # slow DVE reciprocal runs ONCE per jq on [6, TQ] (free
                    # size, not partition count, sets reciprocal cost).
                    oun = onp.tile([128, 2, TQ], F32, tag="oun")
                    nc.scalar.copy(oun[0:HD + 1], o_ps[0:HD + 1])
                    nc.sync.dma_start(rs6_t[2 * p:2 * p + 2, :],
                                      oun[HD:HD + 1, :, :])
                    ouns.append(oun)
                return o_sb, ouns

            def outproj(jq, o_sb, ouns):
                # normalization runs here, AFTER proj(jt+1) on the PE queue,
                # so the rowsum-gather -> reciprocal chain hides under the
                # projection matmuls instead of stalling bc.
                qs = slice(jq * TQ, (jq + 1) * TQ)
                with nc.allow_low_precision(reason="f32r storage is 32-bit"):
                    nc.vector.reciprocal(rinv6_t, rs6_t)
                for p in range(NPAIR):
                    bc = flx.tile([128, TQ], F32, tag="flex")
                    nc.tensor.matmul(bc, e6_t[:, p * 128:(p + 1) * 128],
                                     rinv6_t, start=True, stop=True)
                    oun = ouns[p]
                    nc.vector.tensor_tensor(
                        o_sb[0:64, p], oun[0:64, 0], bc[0:64], MUL)
                    nc.vector.tensor_tensor(
                        o_sb[64:128, p], oun[0:64, 1], bc[64:128], MUL)
                for dt in range(C // 128):
                    po = flx.tile([128, TQ], F32, tag="flex")
                    for p in range(NPAIR):
                        nc.tensor.matmul(
                            po, wo_t[:, p, dt * 128:(dt + 1) * 128],
                            o_sb[:, p], start=(p == 0), stop=(p == NPAIR - 1))
                    ost = msc.tile([128, TQ], F32, tag="ost")
                    nc.vector.tensor_copy(ost, po)
                    nc.sync.dma_start(
                        out.rearrange("(do p) t -> do p t", p=128)[dt, :, qs], ost)

            proj(0)
            for jt in range(NTQ):
                o_sb, ouns = attn(jt)
                if jt + 1 < NTQ:
                    proj(jt + 1)
                outproj(jt, o_sb, ouns)

    nc.compile()
    return nc


def _host_inputs(x, w_qkv, w_out):
    """Build per-core input dicts. Core i: batch i//2, head-group i%2."""
    import ml_dtypes

    BF = ml_dtypes.bfloat16
    xf = np.asarray(x, dtype=np.float32)
    w3 = np.asarray(w_qkv, dtype=np.float32).reshape(3, NH, HD, C)
    wo = np.asarray(w_out, dtype=np.float32)

    per_group = []
    for g in range(2):
        hs = range(g * NHL, (g + 1) * NHL)
        A_ORDER = [4, 5, 6, 7, 8, 9, 10, 11, 0, 1, 2, 3]
        rows = []
        # M-tile 0: r1 (A_ORDER tensor-heads x dims 0:8) + r2a (last 4 of
        # A_ORDER x dims 8:16); M-tile 1 rows 0:64: r2b (first 8 x 8:16)
        for a in A_ORDER:
            tn, hl = divmod(a, NHL)
            rows.append(w3[tn, g * NHL + hl, 0:8])
        for a in A_ORDER[8:12]:
            tn, hl = divmod(a, NHL)
            rows.append(w3[tn, g * NHL + hl, 8:16])
        for a in A_ORDER[0:8]:
            tn, hl = divmod(a, NHL)
            rows.append(w3[tn, g * NHL + hl, 8:16])
        # pass rows: blocks in BLK_ORDER; per blk h_even, h_odd
        for blk in (0, 3, 1, 4, 2, 5):
            tn, pr = divmod(blk, NPAIR)
            for ho in range(2):
                rows.append(w3[tn, g * NHL + 2 * pr + ho, 16:64])
        wqk = np.concatenate(rows, axis=0)                  # [768, C]
        wqkt = np.ascontiguousarray(wqk.T).astype(BF)       # [C, 768]
        wv = w3[2, list(hs)].reshape(CL, C)                 # [384, C]
        wvt = np.ascontiguousarray(wv.T).astype(BF)
        wotr = np.ascontiguousarray(
            wo[:, g * CL:(g + 1) * CL].T).astype(BF)        # [384, 768]
        per_group.append((wqkt, wvt, wotr))

    j = np.arange(RD // 2, dtype=np.float64)
    freqs = 1.0 / (10000.0 ** (2 * j / RD))
    t = np.arange(T, dtype=np.float64)
    ang = t[None, :] * freqs[:, None]                        # [8, T]
    cosb = np.ascontiguousarray(np.tile(np.cos(ang), (12, 1)), dtype=np.float32)
    sinb = np.ascontiguousarray(np.tile(np.sin(ang), (12, 1)), dtype=np.float32)

    kk = np.arange(128)[:, None]
    qq = np.arange(128)[None, :]
    tri = (kk <= qq).astype(BF)
    tri2 = np.ascontiguousarray(np.concatenate([tri, tri], axis=1))
    e6 = np.zeros((6, NPAIR * 128), dtype=np.float32)
    for p in range(NPAIR):
        e6[2 * p, p * 128:p * 128 + 64] = 1.0
        e6[2 * p + 1, p * 128 + 64:(p + 1) * 128] = 1.0

    in_maps = []
    for i in range(8):
        b, g = divmod(i, 2)
        wqkt, wvt, wotr = per_group[g]
        in_maps.append({
            "xt": np.ascontiguousarray(xf[b].T).astype(BF),
            "wqkt": wqkt, "wvt": wvt, "wot": wotr,
            "cosb": cosb, "sinb": sinb, "tri2": tri2, "e6": e6,
        })
    return in_maps


def kernel(x, w_qkv, w_out, _trace=False):
    from concourse.bass_utils import run_bass_kernel_spmd

    if "nc" not in _cache:
        _cache["nc"] = _build()
    nc = _cache["nc"]
    in_maps = _host_inputs(x, w_qkv, w_out)
    res = run_bass_kernel_spmd(nc, in_maps, core_ids=list(range(8)),
                               trace=_trace)
    _cache["last_result"] = res
    out = np.empty((B, T, C), dtype=np.float32)
    for b in range(B):
        acc = res.results[2 * b]["out"].astype(np.float32) + \
            res.results[2 * b + 1]["out"].astype(np.float32)
        out[b] = acc.T
    return out


# revision 9
# speedup vs baseline: 1.2676x; 1.0487x over previous
"""Causal multi-head attention block (qkv proj + partial RoPE + causal attn +
out proj) for Trainium2, distributed over 8 NeuronCores.

Sharding: core i handles batch b = i//2 and head-group g = i%2 (6 of 12 heads).
Each core computes a partial output projection (contraction over its 6 heads'
384 channels); the host sums the two head-group partials per batch.

v3 design notes (from trace analysis of v2 @336us):
  - PE matmul was 99.5% busy: the kernel is tensor-engine bound. Matmul cost
    on HW = ~173ns SBUF access latency (mostly hidden when back-to-back)
    + cols * 0.417ns. fp32r runs ~1.25 cyc/col and triggers power throttling
    (31% of time capped at 50% util) -> all matmuls now bf16 (1 cyc/col).
  - qk projection merged from 7 M-tiles to 6 (rope r1/r2 rows packed with
    pass rows into full 128-row tiles) - fewer streamed columns.
  - Phases interleaved: attn(jq=jt) -> proj(jt+1) -> outproj(jq=jt), so the
    out-proj normalization latency hides under proj matmuls and PE gaps fill.
  - Attention inner loop software-pipelined with lag 2: scores(kt) issue two
    k-tiles ahead of av(kt) so PE never waits on ACT exp.
  - Softmax normalization per pair without cross-pair gather: denominator row
    (ones-column of v) stays in oun; DVE reciprocal on the single-partition
    slice; broadcast to 128 partitions via two K=1 matmuls (e2); DVE multiply
    into bf16 o_sb consumed by the out-projection.
  - DMA order: x(0) first, then weights chunk-by-chunk; x(jt+1) prefetched in
    a single rearranged DMA at the start of attention(jt).
"""

import numpy as np

B, T, C = 4, 2048, 768
NH, HD, RD = 12, 64, 16
NHL = NH // 2          # heads per core (local)
NPAIR = NHL // 2       # head pairs per core
CL = NHL * HD          # local channels (384)
TQ = 512               # q tile
NTQ = T // TQ
NKT = T // 128         # k tiles of 128

_cache = {}


def _build(debug=False):
    import concourse.bacc as bacc
    import concourse.mybir as mybir
    import concourse.tile as tile

    F32R = mybir.dt.float32r
    F32 = mybir.dt.float32
    BF16 = mybir.dt.bfloat16
    AF = mybir.ActivationFunctionType
    MUL = mybir.AluOpType.mult
    SUB = mybir.AluOpType.subtract
    ADD = mybir.AluOpType.add

    nc = bacc.Bacc(trn_type="TRN2", name="attn8v3")

    xt = nc.dram_tensor("xt", [C, T], BF16, kind="ExternalInput")
    wqkt = nc.dram_tensor("wqkt", [C, 2 * CL], BF16, kind="ExternalInput")
    wvt = nc.dram_tensor("wvt", [C, CL], BF16, kind="ExternalInput")
    wot = nc.dram_tensor("wot", [CL, C], BF16, kind="ExternalInput")
    cosb = nc.dram_tensor("cosb", [96, T], BF16, kind="ExternalInput")
    sinb = nc.dram_tensor("sinb", [96, T], BF16, kind="ExternalInput")
    tri2 = nc.dram_tensor("tri2", [128, 2 * 128], BF16, kind="ExternalInput")
    e6 = nc.dram_tensor("e6", [6, NPAIR * 128], F32R, kind="ExternalInput")
    out = nc.dram_tensor("out", [C, T], F32, kind="ExternalOutput")

    # qk-projection M-tiles (wqkt column order, host-built):
    #   tile 0 [128] : r1 rows [96] = (q h0..h5 | k h0..h5) x dims 0:8
    #                  + r2a rows [32] = tensor-heads 0..3 x dims 8:16
    #   tile 1 [128] : r2b rows [64] = tensor-heads 4..11 x dims 8:16
    #                  + pass rows 0:64
    #   tiles 2..5   : pass rows 64:576
    # pass row order: for blk 0..5 (q pairs then k pairs):
    #   h_even dims 16:64 (48 rows), h_odd dims 16:64 (48 rows)
    A_ORDER = [0, 2, 4, 1, 3, 5, 6, 8, 10, 7, 9, 11]

    BLK_ORDER = [0, 3, 1, 4, 2, 5]

    def pass_dest(row):
        g, r = divmod(row, 96)
        part = 64 * (r // 48) + 16 + (r % 48)
        return BLK_ORDER[g], part

    with tile.TileContext(nc) as tc:
        with (
            tc.tile_pool(name="persist", bufs=1) as pp,
            tc.tile_pool(name="weights", bufs=1) as wp,
            tc.tile_pool(name="xload", bufs=2) as xlp,
            tc.tile_pool(name="pstage", bufs=2) as psg,
            tc.tile_pool(name="ropet", bufs=1) as rtp,
            tc.tile_pool(name="rots", bufs=2) as rop,
            tc.tile_pool(name="expp", bufs=3) as xpp,
            tc.tile_pool(name="ounp", bufs=3) as onp,
            tc.tile_pool(name="rinvp", bufs=2) as rip,
            tc.tile_pool(name="osbp", bufs=2) as osp,
            tc.tile_pool(name="misc", bufs=2) as msc,
            tc.tile_pool(name="flex", bufs=2, space="PSUM") as flx,
            tc.tile_pool(name="sps", bufs=2, space="PSUM") as sps,
            tc.tile_pool(name="ops", bufs=1, space="PSUM") as ops,
        ):
            qk_sb = pp.tile([128, 2 * NPAIR, T], BF16, tag="qk")
            v_sb = pp.tile([128, NKT, NHL, HD + 1], BF16, tag="v")
            cos_t = pp.tile([96, T], BF16, tag="cos")
            sin_t = pp.tile([96, T], BF16, tag="sin")
            tri_t = pp.tile([128, 2 * 128], BF16, tag="tri")
            e6_t = pp.tile([6, NPAIR * 128], F32R, tag="e6")
            rs6_t = pp.tile([6, TQ], F32, tag="rs6")
            rinv6_t = pp.tile([6, TQ], F32R, tag="rinv6")

            wqk_t = wp.tile([128, C // 128, 2 * CL], BF16, tag="wqk")
            wv_t = wp.tile([128, C // 128, CL], BF16, tag="wv")
            wo_t = wp.tile([128, NPAIR, C], BF16, tag="wo")

            xr = xt.rearrange("(c p) t -> p c t", p=128)
            wqk_r = wqkt.rearrange("(co p) m -> co p m", p=128)

            def dma_x(jt):
                x_jt = xlp.tile([128, C // 128, TQ], BF16, tag="x")
                ts = slice(jt * TQ, (jt + 1) * TQ)
                nc.scalar.dma_start(x_jt[:, 0:3], xr[:, 0:3, ts])
                nc.scalar.dma_start(x_jt[:, 3:6], xr[:, 3:6, ts])
                return x_jt

            # ---- prologue DMAs: x(0) first, then weights ----
            x_tiles = [dma_x(0)]
            for c in range(C // 128):
                nc.scalar.dma_start(wqk_t[:, c], wqk_r[c])
            nc.scalar.dma_start(
                wv_t, wvt.rearrange("(co p) m -> p co m", p=128))
            nc.gpsimd.memset(
                v_sb.bitcast(mybir.dt.uint16).rearrange("p a b c -> p (a b c)"),
                0x3F80)  # bf16 1.0 bit pattern

            def proj(jt):
                """qkv projections for t-tile jt into qk_sb / v_sb."""
                if jt + 1 < NTQ:
                    x_tiles.append(dma_x(jt + 1))
                ts = slice(jt * TQ, (jt + 1) * TQ)
                x_jt = x_tiles[jt]
                # rope M-tiles (0, 1) share one [128,2,TQ] psum from ops pool
                rps = ops.tile([128, 2, TQ], F32, tag="o")
                for mt in range(2):
                    for c in range(C // 128):
                        nc.tensor.matmul(
                            rps[:, mt], wqk_t[:, c, 128 * mt:128 * (mt + 1)],
                            x_jt[:, c], start=(c == 0), stop=(c == C // 128 - 1))
                if jt == 0:
                    nc.scalar.dma_start(cos_t, cosb[:, :])
                    nc.scalar.dma_start(sin_t, sinb[:, :])
                # rope: rot1 = r1*cos - r2*sin ; rot2 = r2*cos + r1*sin
                # rope rows use A_ORDER = tensor-heads [4..11, 0..3] so the
                # r2 psum split lands on the 64-partition boundary (DVE APs
                # must be 0/32/64/96-aligned): r1 = rps[0:96, 0];
                # r2 = rps[0:64, 1] (heads 4..11) ++ rps[96:128, 0] (0..3)
                t1 = rtp.tile([96, TQ], F32, tag="t1")
                t2 = rtp.tile([96, TQ], F32, tag="t2")
                rot = rop.tile([96, 2, TQ], BF16, tag="rot")
                rot1 = rot[:, 0, :]
                rot2 = rot[:, 1, :]
                nc.vector.tensor_tensor(t1, rps[0:96, 0], cos_t[:, ts], MUL)
                nc.vector.tensor_tensor(
                    t2[0:64], rps[0:64, 1], sin_t[0:64, ts], MUL)
                nc.vector.tensor_tensor(
                    t2[64:96], rps[96:128, 0], sin_t[64:96, ts], MUL)
                nc.vector.tensor_tensor(rot1, t1, t2, SUB)
                t3 = rtp.tile([96, TQ], F32, tag="t1")
                t4 = rtp.tile([96, TQ], F32, tag="t2")
                nc.vector.tensor_tensor(
                    t3[0:64], rps[0:64, 1], cos_t[0:64, ts], MUL)
                nc.vector.tensor_tensor(
                    t3[64:96], rps[96:128, 0], cos_t[64:96, ts], MUL)
                nc.vector.tensor_tensor(t4, rps[0:96, 0], sin_t[:, ts], MUL)
                nc.vector.tensor_tensor(rot2, t3, t4, ADD)
                # rope scatter: one DMA per tensor-head; src [8, 2, TQ]
                # free dims flatten into 16 dst partitions, interleaving
                # (r1_j, r2_j) pairs within the head's rope dims.
                for i, a in enumerate(A_ORDER):
                    tn, hl = divmod(a, NHL)
                    blk = (0 if tn == 0 else NPAIR) + hl // 2
                    base = 64 * (hl % 2)
                    nc.sync.dma_start(qk_sb[base:base + 16, blk, ts],
                                      rot[8 * i:8 * i + 8, :, :])
                if jt == 0:
                    nc.scalar.dma_start(tri_t, tri2[:, :])
                    nc.scalar.dma_start(e6_t, e6[:, :])
                    nc.scalar.dma_start(
                        wo_t, wot.rearrange("(po p) m -> p po m", p=128))
                # pass rows: tile 1 rows 64:128 and tiles 2..5
                for mt in range(1, 6):
                    r0 = 0 if mt > 1 else 64          # first pass row in tile
                    prow0 = 128 * mt + r0 - 192       # pass-row index of r0
                    if mt > 1:
                        ps = flx.tile([128, TQ], F32, tag="flex")
                        for c in range(C // 128):
                            nc.tensor.matmul(
                                ps, wqk_t[:, c, 128 * mt:128 * (mt + 1)],
                                x_jt[:, c], start=(c == 0),
                                stop=(c == C // 128 - 1))
                        src = ps
                    else:
                        src = rps[:, 1]
                    stg = psg.tile([128, TQ], BF16, tag="pstg")
                    nc.vector.tensor_copy(stg[r0:128], src[r0:128])
                    row = prow0
                    while row < prow0 + 128 - r0:
                        blk, part = pass_dest(row)
                        run = min(prow0 + 128 - r0 - row, 48 - row % 48)
                        sr = row - prow0 + r0
                        nc.sync.dma_start(
                            qk_sb[part:part + run, blk, ts],
                            stg[sr:sr + run])
                        row += run
                # v projection: pv[tok, chan] per 128-token chunk
                for vt in range(TQ // 128):
                    pvf = flx.tile([128, TQ], F32, tag="flex")
                    pv = pvf[:, 0:CL]
                    kt0 = jt * (TQ // 128) + vt
                    for c in range(C // 128):
                        nc.tensor.matmul(
                            pv, x_jt[:, c, vt * 128:(vt + 1) * 128],
                            wv_t[:, c], start=(c == 0), stop=(c == C // 128 - 1))
                    nc.vector.tensor_copy(
                        v_sb[:, kt0, :, 0:HD],
                        pv.rearrange("p (h d) -> p h d", d=HD))

            def attn(jq):
                """causal attention for q-tile jq; writes o_sb (bf16)."""
                qs = slice(jq * TQ, (jq + 1) * TQ)
                o_sb = osp.tile([128, NPAIR, TQ], BF16, tag="osb")
                oun_all = onp.tile([128, NPAIR, 2, TQ], F32, tag="oun")
                nkt = 4 * (jq + 1)
                for p in range(NPAIR):
                    qb = qk_sb[:, p, qs]
                    kb = qk_sb[:, NPAIR + p, :]
                    o_ps = ops.tile([128, 2, TQ], F32, tag="o")
                    pend = []

                    def issue_av(kt, ep, a):
                        for h in range(2):
                            nc.tensor.matmul(
                                o_ps[0:HD + 1, h, a:TQ],
                                v_sb[:, kt, 2 * p + h, :], ep[:, h, a:TQ],
                                start=(kt == 0), stop=(kt == nkt - 1))

                    for kt in range(nkt):
                        m = kt - 4 * jq
                        a = 0 if m < 0 else 128 * m
                        ks = slice(kt * 128, (kt + 1) * 128)
                        sg = sps.tile([128, 2, TQ], F32, tag="s")
                        nc.tensor.matmul(
                            sg[:, 0, a:TQ], kb[0:64, ks], qb[0:64, a:TQ],
                            start=True, stop=True, tile_position=(0, 0))
                        nc.tensor.matmul(
                            sg[:, 1, a:TQ], kb[64:128, ks], qb[64:128, a:TQ],
                            start=True, stop=True, tile_position=(64, 0))
                        ep = xpp.tile([128, 2, TQ], BF16, tag="e")
                        nc.scalar.activation(ep[:, :, a:TQ], sg[:, :, a:TQ],
                                             AF.Exp, scale=0.125)
                        if m >= 0:
                            nc.gpsimd.tensor_tensor(
                                ep[:, :, a:a + 128], ep[:, :, a:a + 128],
                                tri_t.rearrange("p (h q) -> p h q", h=2), MUL)
                        pend.append((kt, ep, a))
                        if len(pend) > 2:
                            issue_av(*pend.pop(0))
                    for job in pend:
                        issue_av(*job)
                    # softmax rowsums (ones-column of v) -> oun row 64;
                    # gather to rs6 per pair (cheap 2-descriptor DMA) so the
                    # slow DVE reciprocal runs ONCE per jq on [6, TQ] (free
                    # size, not partition count, sets reciprocal cost).
                    nc.scalar.copy(oun_all[0:HD + 1, p], o_ps[0:HD + 1])
                nc.sync.dma_start(rs6_t, oun_all[HD:HD + 1, :, :, :])
                return o_sb, oun_all

            def outproj(jq, o_sb, oun_all):
                # normalization runs here, AFTER proj(jt+1) on the PE queue,
                # so the rowsum-gather -> reciprocal chain hides under the
                # projection matmuls instead of stalling bc.
                qs = slice(jq * TQ, (jq + 1) * TQ)
                with nc.allow_low_precision(reason="f32r storage is 32-bit"):
                    nc.vector.reciprocal(rinv6_t, rs6_t)
                for p in range(NPAIR):
                    bc = flx.tile([128, TQ], F32, tag="flex")
                    nc.tensor.matmul(bc, e6_t[:, p * 128:(p + 1) * 128],
                                     rinv6_t, start=True, stop=True)
                    nc.vector.tensor_tensor(
                        o_sb[0:64, p], oun_all[0:64, p, 0], bc[0:64], MUL)
                    nc.vector.tensor_tensor(
                        o_sb[64:128, p], oun_all[0:64, p, 1], bc[64:128], MUL)
                ost = msc.tile([128, C // 128, TQ], F32, tag="ost")
                for dt in range(C // 128):
                    po = flx.tile([128, TQ], F32, tag="flex")
                    for p in range(NPAIR):
                        nc.tensor.matmul(
                            po, wo_t[:, p, dt * 128:(dt + 1) * 128],
                            o_sb[:, p], start=(p == 0), stop=(p == NPAIR - 1))
                    nc.vector.tensor_copy(ost[:, dt], po)
                nc.sync.dma_start(
                    out.rearrange("(do p) t -> p do t", p=128)[:, :, qs], ost)

            proj(0)
            for jt in range(NTQ):
                o_sb, oun_all = attn(jt)
                if jt + 1 < NTQ:
                    proj(jt + 1)
                outproj(jt, o_sb, oun_all)

    nc.compile()
    return nc


def _host_inputs(x, w_qkv, w_out):
    """Build per-core input dicts. Core i: batch i//2, head-group i%2."""
    import ml_dtypes

    BF = ml_dtypes.bfloat16
    xf = np.asarray(x, dtype=np.float32)
    w3 = np.asarray(w_qkv, dtype=np.float32).reshape(3, NH, HD, C)
    wo = np.asarray(w_out, dtype=np.float32)

    per_group = []
    for g in range(2):
        hs = range(g * NHL, (g + 1) * NHL)
        A_ORDER = [0, 2, 4, 1, 3, 5, 6, 8, 10, 7, 9, 11]
        rows = []
        # M-tile 0: r1 (A_ORDER tensor-heads x dims 0:8) + r2a (last 4 of
        # A_ORDER x dims 8:16); M-tile 1 rows 0:64: r2b (first 8 x 8:16)
        for a in A_ORDER:
            tn, hl = divmod(a, NHL)
            rows.append(w3[tn, g * NHL + hl, 0:8])
        for a in A_ORDER[8:12]:
            tn, hl = divmod(a, NHL)
            rows.append(w3[tn, g * NHL + hl, 8:16])
        for a in A_ORDER[0:8]:
            tn, hl = divmod(a, NHL)
            rows.append(w3[tn, g * NHL + hl, 8:16])
        # pass rows: blocks in BLK_ORDER; per blk h_even, h_odd
        for blk in (0, 3, 1, 4, 2, 5):
            tn, pr = divmod(blk, NPAIR)
            for ho in range(2):
                rows.append(w3[tn, g * NHL + 2 * pr + ho, 16:64])
        wqk = np.concatenate(rows, axis=0)                  # [768, C]
        wqkt = np.ascontiguousarray(wqk.T).astype(BF)       # [C, 768]
        wv = w3[2, list(hs)].reshape(CL, C)                 # [384, C]
        wvt = np.ascontiguousarray(wv.T).astype(BF)
        wotr = np.ascontiguousarray(
            wo[:, g * CL:(g + 1) * CL].T).astype(BF)        # [384, 768]
        per_group.append((wqkt, wvt, wotr))

    j = np.arange(RD // 2, dtype=np.float64)
    freqs = 1.0 / (10000.0 ** (2 * j / RD))
    t = np.arange(T, dtype=np.float64)
    ang = t[None, :] * freqs[:, None]                        # [8, T]
    cosb = np.ascontiguousarray(np.tile(np.cos(ang), (12, 1))).astype(BF)
    sinb = np.ascontiguousarray(np.tile(np.sin(ang), (12, 1))).astype(BF)

    kk = np.arange(128)[:, None]
    qq = np.arange(128)[None, :]
    tri = (kk <= qq).astype(BF)
    tri2 = np.ascontiguousarray(np.concatenate([tri, tri], axis=1))
    e6 = np.zeros((6, NPAIR * 128), dtype=np.float32)
    for p in range(NPAIR):
        e6[2 * p, p * 128:p * 128 + 64] = 1.0
        e6[2 * p + 1, p * 128 + 64:(p + 1) * 128] = 1.0

    in_maps = []
    for i in range(8):
        b, g = divmod(i, 2)
        wqkt, wvt, wotr = per_group[g]
        in_maps.append({
            "xt": np.ascontiguousarray(xf[b].T).astype(BF),
            "wqkt": wqkt, "wvt": wvt, "wot": wotr,
            "cosb": cosb, "sinb": sinb, "tri2": tri2, "e6": e6,
        })
    return in_maps


def kernel(x, w_qkv, w_out, _trace=False):
    from concourse.bass_utils import run_bass_kernel_spmd

    if "nc" not in _cache:
        _cache["nc"] = _build()
    nc = _cache["nc"]
    in_maps = _host_inputs(x, w_qkv, w_out)
    res = run_bass_kernel_spmd(nc, in_maps, core_ids=list(range(8)),
                               trace=_trace)
    _cache["last_result"] = res
    out = np.empty((B, T, C), dtype=np.float32)
    for b in range(B):
        acc = res.results[2 * b]["out"].astype(np.float32) + \
            res.results[2 * b + 1]["out"].astype(np.float32)
        out[b] = acc.T
    return out


# revision 11
# speedup vs baseline: 1.2817x; 1.0112x over previous
"""Causal multi-head attention block (qkv proj + partial RoPE + causal attn +
out proj) for Trainium2, distributed over 8 NeuronCores.

Sharding: core i handles batch b = i//2 and head-group g = i%2 (6 of 12 heads).
Each core computes a partial output projection (contraction over its 6 heads'
384 channels); the host sums the two head-group partials per batch.

v3 design notes (from trace analysis of v2 @336us):
  - PE matmul was 99.5% busy: the kernel is tensor-engine bound. Matmul cost
    on HW = ~173ns SBUF access latency (mostly hidden when back-to-back)
    + cols * 0.417ns. fp32r runs ~1.25 cyc/col and triggers power throttling
    (31% of time capped at 50% util) -> all matmuls now bf16 (1 cyc/col).
  - qk projection merged from 7 M-tiles to 6 (rope r1/r2 rows packed with
    pass rows into full 128-row tiles) - fewer streamed columns.
  - Phases interleaved: attn(jq=jt) -> proj(jt+1) -> outproj(jq=jt), so the
    out-proj normalization latency hides under proj matmuls and PE gaps fill.
  - Attention inner loop software-pipelined with lag 2: scores(kt) issue two
    k-tiles ahead of av(kt) so PE never waits on ACT exp.
  - Softmax normalization per pair without cross-pair gather: denominator row
    (ones-column of v) stays in oun; DVE reciprocal on the single-partition
    slice; broadcast to 128 partitions via two K=1 matmuls (e2); DVE multiply
    into bf16 o_sb consumed by the out-projection.
  - DMA order: x(0) first, then weights chunk-by-chunk; x(jt+1) prefetched in
    a single rearranged DMA at the start of attention(jt).
"""

import numpy as np

B, T, C = 4, 2048, 768
NH, HD, RD = 12, 64, 16
NHL = NH // 2          # heads per core (local)
NPAIR = NHL // 2       # head pairs per core
CL = NHL * HD          # local channels (384)
TQ = 512               # q tile
NTQ = T // TQ
NKT = T // 128         # k tiles of 128

_cache = {}


def _build(debug=False):
    import concourse.bacc as bacc
    import concourse.mybir as mybir
    import concourse.tile as tile

    F32R = mybir.dt.float32r
    F32 = mybir.dt.float32
    BF16 = mybir.dt.bfloat16
    AF = mybir.ActivationFunctionType
    MUL = mybir.AluOpType.mult
    SUB = mybir.AluOpType.subtract
    ADD = mybir.AluOpType.add

    nc = bacc.Bacc(trn_type="TRN2", name="attn8v3")

    xt = nc.dram_tensor("xt", [C, T], BF16, kind="ExternalInput")
    wqkt = nc.dram_tensor("wqkt", [C, 2 * CL], BF16, kind="ExternalInput")
    wvt = nc.dram_tensor("wvt", [C, CL], BF16, kind="ExternalInput")
    wot = nc.dram_tensor("wot", [CL, C], BF16, kind="ExternalInput")
    cosb = nc.dram_tensor("cosb", [96, T], BF16, kind="ExternalInput")
    sinb = nc.dram_tensor("sinb", [96, T], BF16, kind="ExternalInput")
    tri2 = nc.dram_tensor("tri2", [128, 2 * 128], BF16, kind="ExternalInput")
    e6 = nc.dram_tensor("e6", [6, NPAIR * 128], F32R, kind="ExternalInput")
    out = nc.dram_tensor("out", [C, T], F32, kind="ExternalOutput")

    # qk-projection M-tiles (wqkt column order, host-built):
    #   tile 0 [128] : r1 rows [96] = (q h0..h5 | k h0..h5) x dims 0:8
    #                  + r2a rows [32] = tensor-heads 0..3 x dims 8:16
    #   tile 1 [128] : r2b rows [64] = tensor-heads 4..11 x dims 8:16
    #                  + pass rows 0:64
    #   tiles 2..5   : pass rows 64:576
    # pass row order: for blk 0..5 (q pairs then k pairs):
    #   h_even dims 16:64 (48 rows), h_odd dims 16:64 (48 rows)
    A_ORDER = [0, 2, 4, 1, 3, 5, 6, 8, 10, 7, 9, 11]

    BLK_ORDER = [0, 3, 1, 4, 2, 5]

    def pass_dest(row):
        g, r = divmod(row, 96)
        part = 64 * (r // 48) + 16 + (r % 48)
        return BLK_ORDER[g], part

    with tile.TileContext(nc) as tc:
        with (
            tc.tile_pool(name="persist", bufs=1) as pp,
            tc.tile_pool(name="weights", bufs=1) as wp,
            tc.tile_pool(name="xload", bufs=2) as xlp,
            tc.tile_pool(name="pstage", bufs=2) as psg,
            tc.tile_pool(name="ropet", bufs=1) as rtp,
            tc.tile_pool(name="rots", bufs=2) as rop,
            tc.tile_pool(name="expp", bufs=3) as xpp,
            tc.tile_pool(name="ounp", bufs=3) as onp,
            tc.tile_pool(name="rinvp", bufs=2) as rip,
            tc.tile_pool(name="osbp", bufs=2) as osp,
            tc.tile_pool(name="misc", bufs=2) as msc,
            tc.tile_pool(name="flex", bufs=2, space="PSUM") as flx,
            tc.tile_pool(name="sps", bufs=2, space="PSUM") as sps,
            tc.tile_pool(name="ops", bufs=1, space="PSUM") as ops,
        ):
            qk_sb = pp.tile([128, 2 * NPAIR, T], BF16, tag="qk")
            v_sb = pp.tile([128, NKT, NHL, HD + 1], BF16, tag="v")
            cos_t = pp.tile([96, T], BF16, tag="cos")
            sin_t = pp.tile([96, T], BF16, tag="sin")
            tri_t = pp.tile([128, 2 * 128], BF16, tag="tri")
            e6_t = pp.tile([6, NPAIR * 128], F32R, tag="e6")
            rs6_t = pp.tile([6, TQ], F32, tag="rs6")
            rinv6_t = pp.tile([6, TQ], F32R, tag="rinv6")

            wqk_t = wp.tile([128, C // 128, 2 * CL], BF16, tag="wqk")
            wv_t = wp.tile([128, C // 128, CL], BF16, tag="wv")
            wo_t = wp.tile([128, NPAIR, C], BF16, tag="wo")

            xr = xt.rearrange("(c p) t -> p c t", p=128)
            wqk_r = wqkt.rearrange("(co p) m -> co p m", p=128)

            def dma_x(jt):
                x_jt = xlp.tile([128, C // 128, TQ], BF16, tag="x")
                ts = slice(jt * TQ, (jt + 1) * TQ)
                nc.scalar.dma_start(x_jt[:, 0:3], xr[:, 0:3, ts])
                nc.scalar.dma_start(x_jt[:, 3:6], xr[:, 3:6, ts])
                return x_jt

            # ---- prologue DMAs: x(0) first, then weights ----
            x_tiles = [dma_x(0)]
            for c in range(C // 128):
                nc.scalar.dma_start(wqk_t[:, c], wqk_r[c])
            nc.scalar.dma_start(
                wv_t, wvt.rearrange("(co p) m -> p co m", p=128))
            nc.sync.dma_start(cos_t, cosb[:, :])
            nc.sync.dma_start(sin_t, sinb[:, :])
            nc.sync.dma_start(tri_t, tri2[:, :])
            nc.sync.dma_start(e6_t, e6[:, :])
            nc.gpsimd.memset(
                v_sb.bitcast(mybir.dt.uint16).rearrange("p a b c -> p (a b c)"),
                0x3F80)  # bf16 1.0 bit pattern

            def proj(jt):
                """qkv projections for t-tile jt into qk_sb / v_sb.

                The x(jt+1) prefetch is issued at the END: every later
                consumer of a DGE ring waits on the ring tail at its issue
                time, so an early prefetch would drag the next x transfer
                into this tile's projection critical path."""
                ts = slice(jt * TQ, (jt + 1) * TQ)
                x_jt = x_tiles[jt]
                # rope M-tiles (0, 1) share one [128,2,TQ] psum from ops pool
                rps = ops.tile([128, 2, TQ], F32, tag="o")
                for mt in range(2):
                    for c in range(C // 128):
                        nc.tensor.matmul(
                            rps[:, mt], wqk_t[:, c, 128 * mt:128 * (mt + 1)],
                            x_jt[:, c], start=(c == 0), stop=(c == C // 128 - 1))
                # rope: rot1 = r1*cos - r2*sin ; rot2 = r2*cos + r1*sin
                # rope rows use A_ORDER = tensor-heads [4..11, 0..3] so the
                # r2 psum split lands on the 64-partition boundary (DVE APs
                # must be 0/32/64/96-aligned): r1 = rps[0:96, 0];
                # r2 = rps[0:64, 1] (heads 4..11) ++ rps[96:128, 0] (0..3)
                t1 = rtp.tile([96, TQ], F32, tag="t1")
                t2 = rtp.tile([96, TQ], F32, tag="t2")
                rot = rop.tile([96, 2, TQ], BF16, tag="rot")
                rot1 = rot[:, 0, :]
                rot2 = rot[:, 1, :]
                nc.vector.tensor_tensor(t1, rps[0:96, 0], cos_t[:, ts], MUL)
                nc.vector.tensor_tensor(
                    t2[0:64], rps[0:64, 1], sin_t[0:64, ts], MUL)
                nc.vector.tensor_tensor(
                    t2[64:96], rps[96:128, 0], sin_t[64:96, ts], MUL)
                nc.vector.tensor_tensor(rot1, t1, t2, SUB)
                t3 = rtp.tile([96, TQ], F32, tag="t1")
                t4 = rtp.tile([96, TQ], F32, tag="t2")
                nc.vector.tensor_tensor(
                    t3[0:64], rps[0:64, 1], cos_t[0:64, ts], MUL)
                nc.vector.tensor_tensor(
                    t3[64:96], rps[96:128, 0], cos_t[64:96, ts], MUL)
                nc.vector.tensor_tensor(t4, rps[0:96, 0], sin_t[:, ts], MUL)
                nc.vector.tensor_tensor(rot2, t3, t4, ADD)
                # rope scatter: one DMA per tensor-head; src [8, 2, TQ]
                # free dims flatten into 16 dst partitions, interleaving
                # (r1_j, r2_j) pairs within the head's rope dims.
                for i, a in enumerate(A_ORDER):
                    tn, hl = divmod(a, NHL)
                    blk = (0 if tn == 0 else NPAIR) + hl // 2
                    base = 64 * (hl % 2)
                    nc.sync.dma_start(qk_sb[base:base + 16, blk, ts],
                                      rot[8 * i:8 * i + 8, :, :])
                # pass rows: tile 1 rows 64:128 and tiles 2..5
                for mt in range(1, 6):
                    r0 = 0 if mt > 1 else 64          # first pass row in tile
                    prow0 = 128 * mt + r0 - 192       # pass-row index of r0
                    if mt > 1:
                        ps = flx.tile([128, TQ], F32, tag="flex")
                        for c in range(C // 128):
                            nc.tensor.matmul(
                                ps, wqk_t[:, c, 128 * mt:128 * (mt + 1)],
                                x_jt[:, c], start=(c == 0),
                                stop=(c == C // 128 - 1))
                        src = ps
                    else:
                        src = rps[:, 1]
                    stg = psg.tile([128, TQ], BF16, tag="pstg")
                    nc.vector.tensor_copy(stg[r0:128], src[r0:128])
                    row = prow0
                    while row < prow0 + 128 - r0:
                        blk, part = pass_dest(row)
                        run = min(prow0 + 128 - r0 - row, 48 - row % 48)
                        sr = row - prow0 + r0
                        nc.sync.dma_start(
                            qk_sb[part:part + run, blk, ts],
                            stg[sr:sr + run])
                        row += run
                # v projection: pv[tok, chan] per 128-token chunk
                for vt in range(TQ // 128):
                    pvf = flx.tile([128, TQ], F32, tag="flex")
                    pv = pvf[:, 0:CL]
                    kt0 = jt * (TQ // 128) + vt
                    for c in range(C // 128):
                        nc.tensor.matmul(
                            pv, x_jt[:, c, vt * 128:(vt + 1) * 128],
                            wv_t[:, c], start=(c == 0), stop=(c == C // 128 - 1))
                    nc.vector.tensor_copy(
                        v_sb[:, kt0, :, 0:HD],
                        pv.rearrange("p (h d) -> p h d", d=HD))
                if jt == 0:
                    nc.scalar.dma_start(
                        wo_t, wot.rearrange("(po p) m -> p po m", p=128))
                if jt + 1 < NTQ:
                    x_tiles.append(dma_x(jt + 1))

            def attn(jq):
                """causal attention for q-tile jq; writes o_sb (bf16)."""
                qs = slice(jq * TQ, (jq + 1) * TQ)
                o_sb = osp.tile([128, NPAIR, TQ], BF16, tag="osb")
                oun_all = onp.tile([128, NPAIR, 2, TQ], F32, tag="oun")
                nkt = 4 * (jq + 1)
                for p in range(NPAIR):
                    qb = qk_sb[:, p, qs]
                    kb = qk_sb[:, NPAIR + p, :]
                    o_ps = ops.tile([128, 2, TQ], F32, tag="o")
                    pend = []

                    def issue_av(kt, ep, a):
                        for h in range(2):
                            nc.tensor.matmul(
                                o_ps[0:HD + 1, h, a:TQ],
                                v_sb[:, kt, 2 * p + h, :], ep[:, h, a:TQ],
                                start=(kt == 0), stop=(kt == nkt - 1))

                    for kt in range(nkt):
                        m = kt - 4 * jq
                        a = 0 if m < 0 else 128 * m
                        ks = slice(kt * 128, (kt + 1) * 128)
                        sg = sps.tile([128, 2, TQ], F32, tag="s")
                        nc.tensor.matmul(
                            sg[:, 0, a:TQ], kb[0:64, ks], qb[0:64, a:TQ],
                            start=True, stop=True, tile_position=(0, 0))
                        nc.tensor.matmul(
                            sg[:, 1, a:TQ], kb[64:128, ks], qb[64:128, a:TQ],
                            start=True, stop=True, tile_position=(64, 0))
                        ep = xpp.tile([128, 2, TQ], BF16, tag="e")
                        nc.scalar.activation(ep[:, :, a:TQ], sg[:, :, a:TQ],
                                             AF.Exp, scale=0.125)
                        if m >= 0:
                            nc.gpsimd.tensor_tensor(
                                ep[:, :, a:a + 128], ep[:, :, a:a + 128],
                                tri_t.rearrange("p (h q) -> p h q", h=2), MUL)
                        pend.append((kt, ep, a))
                        if len(pend) > 2:
                            issue_av(*pend.pop(0))
                    for job in pend:
                        issue_av(*job)
                    # softmax rowsums (ones-column of v) -> oun row 64;
                    # gather to rs6 per pair (cheap 2-descriptor DMA) so the
                    # slow DVE reciprocal runs ONCE per jq on [6, TQ] (free
                    # size, not partition count, sets reciprocal cost).
                    nc.scalar.copy(oun_all[0:HD + 1, p], o_ps[0:HD + 1])
                nc.sync.dma_start(rs6_t, oun_all[HD:HD + 1, :, :, :])
                return o_sb, oun_all

            def outproj(jq, o_sb, oun_all):
                # normalization runs here, AFTER proj(jt+1) on the PE queue,
                # so the rowsum-gather -> reciprocal chain hides under the
                # projection matmuls instead of stalling bc.
                qs = slice(jq * TQ, (jq + 1) * TQ)
                with nc.allow_low_precision(reason="f32r storage is 32-bit"):
                    nc.vector.reciprocal(rinv6_t, rs6_t)
                for p in range(NPAIR):
                    bc = flx.tile([128, TQ], F32, tag="flex")
                    nc.tensor.matmul(bc, e6_t[:, p * 128:(p + 1) * 128],
                                     rinv6_t, start=True, stop=True)
                    nc.vector.tensor_tensor(
                        o_sb[0:64, p], oun_all[0:64, p, 0], bc[0:64], MUL)
                    nc.vector.tensor_tensor(
                        o_sb[64:128, p], oun_all[0:64, p, 1], bc[64:128], MUL)
                ost = msc.tile([128, C // 128, TQ], F32, tag="ost")
                for dt in range(C // 128):
                    po = flx.tile([128, TQ], F32, tag="flex")
                    for p in range(NPAIR):
                        nc.tensor.matmul(
                            po, wo_t[:, p, dt * 128:(dt + 1) * 128],
                            o_sb[:, p], start=(p == 0), stop=(p == NPAIR - 1))
                    nc.vector.tensor_copy(ost[:, dt], po)
                nc.sync.dma_start(
                    out.rearrange("(do p) t -> p do t", p=128)[:, :, qs], ost)

            proj(0)
            for jt in range(NTQ):
                o_sb, oun_all = attn(jt)
                if jt + 1 < NTQ:
                    proj(jt + 1)
                outproj(jt, o_sb, oun_all)

    nc.compile()
    return nc


def _host_inputs(x, w_qkv, w_out):
    """Build per-core input dicts. Core i: batch i//2, head-group i%2."""
    import ml_dtypes

    BF = ml_dtypes.bfloat16
    xf = np.asarray(x, dtype=np.float32)
    w3 = np.asarray(w_qkv, dtype=np.float32).reshape(3, NH, HD, C)
    wo = np.asarray(w_out, dtype=np.float32)

    per_group = []
    for g in range(2):
        hs = range(g * NHL, (g + 1) * NHL)
        A_ORDER = [0, 2, 4, 1, 3, 5, 6, 8, 10, 7, 9, 11]
        rows = []
        # M-tile 0: r1 (A_ORDER tensor-heads x dims 0:8) + r2a (last 4 of
        # A_ORDER x dims 8:16); M-tile 1 rows 0:64: r2b (first 8 x 8:16)
        for a in A_ORDER:
            tn, hl = divmod(a, NHL)
            rows.append(w3[tn, g * NHL + hl, 0:8])
        for a in A_ORDER[8:12]:
            tn, hl = divmod(a, NHL)
            rows.append(w3[tn, g * NHL + hl, 8:16])
        for a in A_ORDER[0:8]:
            tn, hl = divmod(a, NHL)
            rows.append(w3[tn, g * NHL + hl, 8:16])
        # pass rows: blocks in BLK_ORDER; per blk h_even, h_odd
        for blk in (0, 3, 1, 4, 2, 5):
            tn, pr = divmod(blk, NPAIR)
            for ho in range(2):
                rows.append(w3[tn, g * NHL + 2 * pr + ho, 16:64])
        wqk = np.concatenate(rows, axis=0)                  # [768, C]
        wqkt = np.ascontiguousarray(wqk.T).astype(BF)       # [C, 768]
        wv = w3[2, list(hs)].reshape(CL, C)                 # [384, C]
        wvt = np.ascontiguousarray(wv.T).astype(BF)
        wotr = np.ascontiguousarray(
            wo[:, g * CL:(g + 1) * CL].T).astype(BF)        # [384, 768]
        per_group.append((wqkt, wvt, wotr))

    j = np.arange(RD // 2, dtype=np.float64)
    freqs = 1.0 / (10000.0 ** (2 * j / RD))
    t = np.arange(T, dtype=np.float64)
    ang = t[None, :] * freqs[:, None]                        # [8, T]
    cosb = np.ascontiguousarray(np.tile(np.cos(ang), (12, 1))).astype(BF)
    sinb = np.ascontiguousarray(np.tile(np.sin(ang), (12, 1))).astype(BF)

    kk = np.arange(128)[:, None]
    qq = np.arange(128)[None, :]
    tri = (kk <= qq).astype(BF)
    tri2 = np.ascontiguousarray(np.concatenate([tri, tri], axis=1))
    e6 = np.zeros((6, NPAIR * 128), dtype=np.float32)
    for p in range(NPAIR):
        e6[2 * p, p * 128:p * 128 + 64] = 1.0
        e6[2 * p + 1, p * 128 + 64:(p + 1) * 128] = 1.0

    in_maps = []
    for i in range(8):
        b, g = divmod(i, 2)
        wqkt, wvt, wotr = per_group[g]
        in_maps.append({
            "xt": np.ascontiguousarray(xf[b].T).astype(BF),
            "wqkt": wqkt, "wvt": wvt, "wot": wotr,
            "cosb": cosb, "sinb": sinb, "tri2": tri2, "e6": e6,
        })
    return in_maps


def kernel(x, w_qkv, w_out, _trace=False):
    from concourse.bass_utils import run_bass_kernel_spmd

    if "nc" not in _cache:
        _cache["nc"] = _build()
    nc = _cache["nc"]
    in_maps = _host_inputs(x, w_qkv, w_out)
    res = run_bass_kernel_spmd(nc, in_maps, core_ids=list(range(8)),
                               trace=_trace)
    _cache["last_result"] = res
    out = np.empty((B, T, C), dtype=np.float32)
    for b in range(B):
        acc = res.results[2 * b]["out"].astype(np.float32) + \
            res.results[2 * b + 1]["out"].astype(np.float32)
        out[b] = acc.T
    return out
